# revision 13
# baseline (speedup 1.0000x reference)
"""Trainium2 Bass kernel for nn_AffinityPredictor (GNN message passing).

Strategy: shard the 32 graphs across 8 cores (4 graphs/core, nodes split at
graph boundaries). Each core:
  - computes a full-graph combined K|V node table in bf16 (edges cross graph
    boundaries) + own-range Q/W2Q/skip tables resident in SBUF,
  - streams its edges (sorted by destination, packed into 128-node blocks
    balanced by degree) through one dma_gather of 1024-wide K|V rows per edge,
  - expands per-edge Q via a one-hot matmul (no q gather), computes edge
    attention incl. the edge-encoder contribution via host-folded wq@we.T
    weights, segment softmax via one-hot scatter matmuls accumulating
    numerator/denominator in PSUM per destination block,
  - applies skip/ELU/projection, sequence-guided attention pooling per graph,
  - runs the small head MLPs for its 4 graphs.
Host only does index preprocessing/sharding/weight folding; all activation
FLOPs run on device.
"""
import sys

if "/opt/trn_rl_repo" not in sys.path:
    sys.path.insert(0, "/opt/trn_rl_repo")

import numpy as np
import ml_dtypes
from contextlib import ExitStack

BF_NP = ml_dtypes.bfloat16

import concourse.bass as bass
import concourse.tile as tile
from concourse import bacc, mybir
from concourse.bass_utils import run_bass_kernel_spmd

F32 = mybir.dt.float32
BF16 = mybir.dt.bfloat16
I16 = mybir.dt.int16
I32 = mybir.dt.int32
AF = mybir.ActivationFunctionType
OP = mybir.AluOpType

NCORES = 8
HEADS, OC, HID = 2, 256, 128
D = HEADS * OC          # 512
GQ = 512                # edges per gather call (4 chunks of 128)
NEG = -1.0e30

_CACHE = {}


# ----------------------------------------------------------------------------
# host-side preprocessing
# ----------------------------------------------------------------------------

def _pack_blocks(own_ids, deg, NB):
    """Greedy: place nodes (desc by degree) into NB bins (<=128 nodes each),
    balancing total edges. Returns list of NB node-id lists."""
    order = np.argsort(-deg, kind="stable")
    bin_e = np.zeros(NB, np.int64)
    bin_n = np.zeros(NB, np.int64)
    blocks = [[] for _ in range(NB)]
    for i in order:
        cand = np.where(bin_n < 128)[0]
        b = cand[np.argmin(bin_e[cand])]
        blocks[b].append(int(own_ids[i]))
        bin_e[b] += int(deg[i])
        bin_n[b] += 1
    return blocks


def _wrap16(vals):
    n = len(vals)
    a = np.asarray(vals, np.int64).reshape(n // 16, 16).T.astype(np.int16)
    return np.tile(a, (8, 1))


def _wrap128(vals, dtype=np.float32):
    n = len(vals)
    return np.ascontiguousarray(np.asarray(vals, dtype).reshape(n // 128, 128).T)


def _prep(inputs):
    p = {k: np.asarray(v, np.float32) for k, v in inputs["params"].items()}
    types = ["L", "H", "A"]
    N = inputs["Lx"].shape[0]
    B = 32
    gpb = B // NCORES

    # ---- pass 1: sizes
    per = {}
    NB = 1
    for t in types:
        batch = np.asarray(inputs[t + "_batch"], np.int64)
        ei = np.asarray(inputs[t + "_edge_index"], np.int64)
        dst = ei[1]
        deg = np.bincount(dst, minlength=N)
        for c in range(NCORES):
            own = np.where((batch >= c * gpb) & (batch < (c + 1) * gpb))[0]
            per[(t, c)] = (own, deg[own])
            NB = max(NB, (len(own) + 127) // 128)

    blocks_all = {}
    CB = 1
    for t in types:
        ei = np.asarray(inputs[t + "_edge_index"], np.int64)
        dst = ei[1]
        deg_full = np.bincount(dst, minlength=N)
        for c in range(NCORES):
            own, dg = per[(t, c)]
            blocks = _pack_blocks(own, dg, NB)
            blocks_all[(t, c)] = blocks
            for bl in blocks:
                cnt = int(deg_full[bl].sum()) if bl else 0
                CB = max(CB, (cnt + 127) // 128)
    CB = ((CB + 3) // 4) * 4  # multiple of 4 so EP % GQ == 0 (NB may be odd)
    NBn = NB * 128
    EP = NB * CB * 128

    # ---- pass 2: materialize per-core arrays
    in_maps = [dict() for _ in range(NCORES)]
    for t in types:
        x = np.asarray(inputs[t + "x"], np.float32)
        ei = np.asarray(inputs[t + "_edge_index"], np.int64)
        ea = np.asarray(inputs[t + "_edge_attr"], np.float32)
        batch = np.asarray(inputs[t + "_batch"], np.int64)
        src, dst = ei[0], ei[1]

        xT_aug = np.ones((24, N), np.float32)
        xT_aug[:23] = x.T
        xT_aug = xT_aug.astype(np.float32)
        core_of_edge = batch[dst] // gpb

        for c in range(NCORES):
            blocks = blocks_all[(t, c)]
            slot = np.full(N, -1, np.int64)
            for b, bl in enumerate(blocks):
                for j, nid in enumerate(bl):
                    slot[nid] = b * 128 + j

            xT_own = np.zeros((24, NBn), np.float32)
            nmask = np.full(NBn, NEG, np.float32)
            ohg = np.zeros((NBn, 4), np.float32)
            ohgT = np.zeros((4, NBn), np.float32)
            own_ids = np.where(slot >= 0)[0]
            s_own = slot[own_ids]
            xT_own[:23, s_own] = x[own_ids].T
            xT_own[23, s_own] = 1.0
            nmask[s_own] = 0.0
            glocal = batch[own_ids] - c * gpb
            ohg[s_own, glocal] = 1.0
            ohgT[glocal, s_own] = 1.0

            emask = core_of_edge == c
            es = src[emask]
            eat = ea[emask]
            eslot = slot[dst[emask]]
            eblock = eslot // 128
            order = np.argsort(eblock, kind="stable")
            es, eat, eslot, eblock = es[order], eat[order], eslot[order], eblock[order]

            src_pad = np.zeros(EP, np.int64)
            dstl_pad = np.zeros(EP, np.float32)
            lmask_pad = np.full(EP, NEG, np.float32)
            eat_pad = np.zeros((EP, 2), np.float32)
            bstart = np.searchsorted(eblock, np.arange(NB))
            bend = np.searchsorted(eblock, np.arange(NB), side="right")
            for b in range(NB):
                s0, s1 = int(bstart[b]), int(bend[b])
                cnt = s1 - s0
                assert cnt <= CB * 128
                o = b * CB * 128
                src_pad[o:o + cnt] = es[s0:s1]
                dstl_pad[o:o + cnt] = (eslot[s0:s1] % 128).astype(np.float32)
                lmask_pad[o:o + cnt] = 0.0
                eat_pad[o:o + cnt] = eat[s0:s1]

            eattrT = np.ones((3, EP), np.float32)
            eattrT[:2] = eat_pad.T

            m = in_maps[c]
            m[t + "_xT"] = xT_aug.astype(BF_NP)
            m[t + "_xTo"] = xT_own.astype(BF_NP)
            m[t + "_srcw"] = _wrap16(src_pad)
            m[t + "_dstl"] = _wrap128(dstl_pad)
            m[t + "_lmask"] = _wrap128(lmask_pad)
            m[t + "_eattrT"] = eattrT.astype(BF_NP)
            m[t + "_nmask"] = np.ascontiguousarray(nmask.reshape(NB, 128).T)
            m[t + "_ohg"] = np.ascontiguousarray(
                ohg.reshape(NB, 128, 4).transpose(1, 0, 2).reshape(128, NB * 4))
            m[t + "_ohgT"] = ohgT

            sq = np.asarray(inputs[t + "s"], np.float32)[c * gpb:(c + 1) * gpb]
            wsq = np.asarray(inputs["W" + t + "s"], np.float32)[c * gpb:(c + 1) * gpb]
            seqs = np.zeros((1408, 8), np.float32)
            seqs[:1280, 0:4] = sq.T
            seqs[:1280, 4:8] = wsq.T
            seqs[1280, :] = 1.0
            m[t + "_seqs"] = seqs

    # ---- shared weights
    w = {}
    bf = BF_NP

    def aug24(wname, bname, dim):
        a = np.zeros((24, dim), np.float32)
        a[:23] = p[wname]
        a[23] = p[bname]
        return a

    wq_aug = aug24("conv_wq", "conv_bq", D)
    we = p["conv_we"]                        # [32, 512]
    # folded q@we.T per head -> [24, 64]
    wq2 = np.concatenate(
        [wq_aug[:, h * OC:(h + 1) * OC] @ we[:, h * OC:(h + 1) * OC].T
         for h in range(2)], axis=1)
    w["wqq"] = np.concatenate([wq_aug, wq2], axis=1).astype(bf)   # [24, 576]
    w["wkv"] = np.concatenate(
        [aug24("conv_wk", "conv_bk", D), aug24("conv_wv", "conv_bv", D)],
        axis=1).astype(bf)                                        # [24, 1024]
    w["wskip"] = aug24("conv_wskip", "conv_bskip", OC).astype(bf)
    eew1 = np.zeros((3, 32), np.float32)
    eew1[:2] = p["ee_w1"]
    eew1[2] = p["ee_b1"]
    w["eew1"] = eew1.astype(bf)
    w["eew2"] = p["ee_w2"].astype(bf)
    w["eeb2"] = p["ee_b2"].reshape(32, 1).astype(np.float32)
    w["we"] = we.astype(bf)
    w["proj1w"] = p["proj1_w"]
    w["proj1b"] = p["proj1_b"].reshape(1, HID).copy()
    w["poolwq"] = p["pool_wq"]
    w["poolwk"] = p["pool_wk"]
    w["poolvB"] = np.tile(p["pool_v"].reshape(1, HID), (128, 1))
    seqw = np.zeros((1408, HID), np.float32)
    seqw[:1280] = p["seq_w"]
    seqw[1280] = p["seq_b"]
    w["seqw"] = seqw
    for pre in ["dg", "ddg", "inter"]:
        w[pre + "w1"] = p[pre + "_w1"]
        w[pre + "b1"] = p[pre + "_b1"].reshape(1, HID).copy()
        w[pre + "w2"] = p[pre + "_w2"]
    b2sum = float(p["dg_b2"][0] + p["inter_b2"][0] + p["ddg_b2"][0])

    meta = dict(N=N, NB=NB, CB=CB, EP=EP, NBn=NBn, b2sum=b2sum)
    return meta, in_maps, w


# ----------------------------------------------------------------------------
# kernel builder
# ----------------------------------------------------------------------------

def _build(meta, w):
    N, NB, CB, EP, NBn = meta["N"], meta["NB"], meta["CB"], meta["EP"], meta["NBn"]
    NT = (N + 127) // 128
    types = ["L", "H", "A"]

    nc = bacc.Bacc("TRN2", target_bir_lowering=False, debug=False,
                   num_devices=NCORES, num_swdge_queues=4)

    def din(name, shape, dt=F32):
        return nc.dram_tensor(name, shape, dt, kind="ExternalInput").ap()

    dd = {}
    for t in types:
        dd[t + "_xT"] = din(t + "_xT", [24, N], BF16)
        dd[t + "_xTo"] = din(t + "_xTo", [24, NBn], BF16)
        dd[t + "_srcw"] = din(t + "_srcw", [128, EP // 16], I16)
        dd[t + "_dstl"] = din(t + "_dstl", [128, EP // 128])
        dd[t + "_lmask"] = din(t + "_lmask", [128, EP // 128])
        dd[t + "_eattrT"] = din(t + "_eattrT", [3, EP], BF16)
        dd[t + "_nmask"] = din(t + "_nmask", [128, NB])
        dd[t + "_ohg"] = din(t + "_ohg", [128, NB * 4])
        dd[t + "_ohgT"] = din(t + "_ohgT", [4, NBn])
        dd[t + "_seqs"] = din(t + "_seqs", [1408, 8])
        dd[t + "_kvtab"] = nc.dram_tensor(t + "_kvtab", [N, 2 * D], BF16).ap()
    for k, v in w.items():
        dt = BF16 if v.dtype == BF_NP else F32
        dd[k] = din("w_" + k, list(v.shape), dt)
    pred_d = nc.dram_tensor("pred", [1, 4], F32, kind="ExternalOutput").ap()

    with tile.TileContext(nc) as tc, ExitStack() as gctx:
        const = gctx.enter_context(tc.tile_pool(name="const", bufs=1))
        wpool = gctx.enter_context(tc.tile_pool(name="wts", bufs=2))
        npool = gctx.enter_context(tc.tile_pool(name="nod", bufs=3))
        gkv = gctx.enter_context(tc.tile_pool(name="gkv", bufs=4))
        ep = gctx.enter_context(tc.tile_pool(name="ep", bufs=3))
        bp = gctx.enter_context(tc.tile_pool(name="bp", bufs=2))
        fin = gctx.enter_context(tc.tile_pool(name="fin", bufs=1))
        psA = gctx.enter_context(tc.tile_pool(name="psA", bufs=3, space="PSUM"))
        psN = gctx.enter_context(tc.tile_pool(name="psN", bufs=2, space="PSUM"))
        psS = gctx.enter_context(tc.tile_pool(name="psS", bufs=3, space="PSUM"))

        # ---- constants
        iota_i = const.tile([128, 128], I32)
        nc.gpsimd.iota(iota_i[:], pattern=[[1, 128]], base=0, channel_multiplier=0)
        iota_f = const.tile([128, 128], F32)
        nc.vector.tensor_copy(iota_f[:], iota_i[:])
        iotac_i = const.tile([128, 1], I32)
        nc.gpsimd.iota(iotac_i[:], pattern=[[1, 1]], base=0, channel_multiplier=1)
        iotac_f = const.tile([128, 1], F32)
        nc.vector.tensor_copy(iotac_f[:], iotac_i[:])
        ident = const.tile([128, 128], F32)
        nc.vector.tensor_scalar(out=ident[:], in0=iota_f[:], scalar1=iotac_f[:],
                                scalar2=None, op0=OP.is_equal)
        ident_bf = const.tile([128, 128], BF16)
        nc.vector.tensor_copy(ident_bf[:], ident[:])
        ones1 = const.tile([1, 128], F32)
        nc.vector.memset(ones1[:], 1.0)

        def wtile(name, pool=const):
            arr = w[name]
            dt = BF16 if arr.dtype == BF_NP else F32
            tl = pool.tile([arr.shape[0], arr.shape[1]], dt, tag="w_" + name)
            nc.sync.dma_start(tl[:], dd[name][:])
            return tl

        wqq_t = wtile("wqq"); wkv_t = wtile("wkv"); wskip_t = wtile("wskip")
        eew1_t = wtile("eew1"); eew2_t = wtile("eew2"); eeb2_t = wtile("eeb2")
        we_t = wtile("we")
        proj1b_t = wtile("proj1b")
        poolwq_t = wtile("poolwq"); poolwk_t = wtile("poolwk")
        poolvB_t = wtile("poolvB")

        G2 = {}
        SEQ = {}

        for t in types:
            with ExitStack() as tctx:
                res = tctx.enter_context(tc.tile_pool(name="res" + t, bufs=1))
                skl = tctx.enter_context(tc.tile_pool(name="skl" + t, bufs=1))

                # ---- seq encoders
                seq_ps = psS.tile([128, 8], F32, tag="s")
                for kc in range(11):
                    wt = wpool.tile([128, HID], F32, tag="seqw")
                    nc.sync.dma_start(wt[:], dd["seqw"][kc * 128:(kc + 1) * 128, :])
                    st = wpool.tile([128, 8], F32, tag="seqs")
                    nc.sync.dma_start(st[:], dd[t + "_seqs"][kc * 128:(kc + 1) * 128, :])
                    nc.tensor.matmul(seq_ps[:], wt[:], st[:], start=(kc == 0),
                                     stop=(kc == 10))
                seq_sb = fin.tile([128, 8], F32, tag="seq" + t)
                nc.vector.tensor_copy(seq_sb[:], seq_ps[:])
                SEQ[t] = seq_sb
                ctxk_ps = psS.tile([4, 128], F32, tag="s")
                nc.tensor.matmul(ctxk_ps[:], seq_sb[:, 0:4], poolwk_t[:],
                                 start=True, stop=True)
                ctxk_sb = res.tile([4, 128], F32, tag="ctxk")
                nc.vector.tensor_copy(ctxk_sb[:], ctxk_ps[:])

                # ---- resident per-type arrays
                def rload(nm, shape, dt=F32):
                    tl = res.tile(shape, dt, tag=nm)
                    nc.sync.dma_start(tl[:], dd[t + "_" + nm][:])
                    return tl

                srcw_t = rload("srcw", [128, EP // 16], I16)
                dstl_t = rload("dstl", [128, EP // 128])
                lmask_t = rload("lmask", [128, EP // 128])
                nmask_t = rload("nmask", [128, NB])
                ohg_t = rload("ohg", [128, NB * 4])
                ohgT_t = rload("ohgT", [4, NBn])

                # ---- node tables: combined K|V (bf16) -> DRAM
                for nt in range(NT):
                    cols = min(128, N - nt * 128)
                    xt = npool.tile([24, 128], BF16, tag="xt")
                    nc.sync.dma_start(xt[:, :cols],
                                      dd[t + "_xT"][:, nt * 128:nt * 128 + cols])
                    kv_sb = npool.tile([128, 2 * D], BF16, tag="kvsb")
                    for half in range(2):
                        ps = psA.tile([128, D], F32, tag="a")
                        nc.tensor.matmul(ps[:cols, :], xt[:, :cols],
                                         wkv_t[:, half * D:(half + 1) * D],
                                         start=True, stop=True)
                        if half == 0:
                            nc.vector.tensor_copy(kv_sb[:cols, 0:D], ps[:cols, :])
                        else:
                            nc.scalar.copy(kv_sb[:cols, D:2 * D], ps[:cols, :])
                    nc.sync.dma_start(
                        dd[t + "_kvtab"][nt * 128:nt * 128 + cols, :],
                        kv_sb[:cols, :])

                # ---- own-range Q|W2Q blocks (SBUF resident, bf16) + skip
                qbs = []
                skips = []
                for nt in range(NB):
                    xt = npool.tile([24, 128], BF16, tag="xt")
                    nc.sync.dma_start(xt[:], dd[t + "_xTo"][:, nt * 128:(nt + 1) * 128])
                    qb = skl.tile([128, 576], BF16, tag="qb" + str(nt))
                    ps = psA.tile([128, D], F32, tag="a")
                    nc.tensor.matmul(ps[:], xt[:], wqq_t[:, 0:D], start=True, stop=True)
                    nc.vector.tensor_copy(qb[:, 0:D], ps[:])
                    ps2 = psS.tile([128, 64], F32, tag="s")
                    nc.tensor.matmul(ps2[:], xt[:], wqq_t[:, D:576], start=True, stop=True)
                    nc.scalar.copy(qb[:, D:576], ps2[:])
                    qbs.append(qb)
                    ps3 = psS.tile([128, OC], F32, tag="s")
                    nc.tensor.matmul(ps3[:], xt[:], wskip_t[:], start=True, stop=True)
                    sk = skl.tile([128, OC], F32, tag="skip" + str(nt))
                    nc.scalar.copy(sk[:], ps3[:])
                    skips.append(sk)

                # ---- pooling accumulators
                pnum = res.tile([128, 4], F32, tag="pnum")
                nc.vector.memset(pnum[:], 0.0)
                pden = res.tile([1, 4], F32, tag="pden")
                nc.vector.memset(pden[:], 0.0)

                # ---- edge phase
                state = {}

                def gather_call(g):
                    kvt = gkv.tile([128, 4, 2 * D], BF16, tag="kv")
                    nc.gpsimd.dma_gather(kvt[:], dd[t + "_kvtab"][:],
                                         srcw_t[:, g * 32:(g + 1) * 32],
                                         num_idxs=GQ, num_idxs_reg=GQ,
                                         elem_size=2 * D, queue_num=g % 4)
                    et = ep.tile([3, GQ], BF16, tag="eat")
                    nc.sync.dma_start(et[:], dd[t + "_eattrT"][:, g * GQ:(g + 1) * GQ])
                    ps1 = psS.tile([32, GQ], F32, tag="s")
                    nc.tensor.matmul(ps1[:], eew1_t[:], et[:], start=True, stop=True)
                    e1 = ep.tile([32, GQ], BF16, tag="enc1")
                    nc.scalar.activation(e1[:], ps1[:], AF.Relu)
                    ps2 = psS.tile([32, GQ], F32, tag="s")
                    nc.tensor.matmul(ps2[:], eew2_t[:], e1[:], start=True, stop=True)
                    e2 = ep.tile([32, GQ], BF16, tag="enc2")
                    nc.scalar.activation(e2[:], ps2[:], AF.Identity, bias=eeb2_t[:])
                    state["kv"], state["e2"] = kvt, e2

                for b in range(NB):
                    num_ps = psN.tile([128, D], F32, tag="n")
                    extra_ps = psS.tile([128, 66], F32, tag="s")
                    for cc in range(CB):
                        gidx = b * CB + cc
                        g, sl = gidx // 4, gidx % 4
                        if sl == 0:
                            gather_call(g)
                        kvt, e2 = state["kv"], state["e2"]
                        e2c = e2[:, sl * 128:(sl + 1) * 128]
                        tr1 = psS.tile([128, 32], BF16, tag="s")
                        nc.tensor.transpose(tr1[:], e2c[:], ident_bf[:32, :32])
                        e2em = ep.tile([128, 32], BF16, tag="e2em", bufs=6)
                        nc.vector.tensor_copy(e2em[:], tr1[:])
                        oh = ep.tile([128, 128], BF16, tag="oh", bufs=6)
                        nc.vector.tensor_scalar(out=oh[:], in0=iota_f[:],
                                                scalar1=dstl_t[:, gidx:gidx + 1],
                                                scalar2=None, op0=OP.is_equal)
                        tr2 = psS.tile([128, 128], BF16, tag="s")
                        nc.tensor.transpose(tr2[:], oh[:], ident_bf[:])
                        ohT = ep.tile([128, 128], BF16, tag="ohT", bufs=6)
                        nc.vector.tensor_copy(ohT[:], tr2[:])
                        qexa = psA.tile([128, D], F32, tag="a")
                        nc.tensor.matmul(qexa[:], ohT[:], qbs[b][:, 0:D],
                                         start=True, stop=True)
                        qexb = psS.tile([128, 64], F32, tag="s")
                        nc.tensor.matmul(qexb[:], ohT[:], qbs[b][:, D:576],
                                         start=True, stop=True)
                        aqk = ep.tile([128, 2], F32, tag="aqk", bufs=6)
                        aee = ep.tile([128, 2], F32, tag="aee", bufs=6)
                        scr = ep.tile([128, OC], F32, tag="scr", bufs=8)
                        scr2 = ep.tile([128, 32], F32, tag="scr32", bufs=8)
                        for h in range(2):
                            nc.vector.scalar_tensor_tensor(
                                out=scr[:], in0=kvt[:, sl, h * OC:(h + 1) * OC],
                                scalar=0.0625, in1=qexa[:, h * OC:(h + 1) * OC],
                                op0=OP.mult, op1=OP.mult,
                                accum_out=aqk[:, h:h + 1])
                            nc.vector.scalar_tensor_tensor(
                                out=scr2[:], in0=e2em[:],
                                scalar=0.0625, in1=qexb[:, h * 32:(h + 1) * 32],
                                op0=OP.mult, op1=OP.mult,
                                accum_out=aee[:, h:h + 1])
                        alpha = ep.tile([128, 2], F32, tag="alpha", bufs=6)
                        nc.vector.scalar_tensor_tensor(
                            out=alpha[:], in0=aqk[:],
                            scalar=lmask_t[:, gidx:gidx + 1], in1=aee[:],
                            op0=OP.add, op1=OP.add)
                        ea = ep.tile([128, 2], F32, tag="ea", bufs=6)
                        nc.scalar.activation(ea[:], alpha[:], AF.Exp)
                        vea = ep.tile([128, D], BF16, tag="vea", bufs=6)
                        combo = ep.tile([128, 66], BF16, tag="combo", bufs=6)
                        for h in range(2):
                            nc.scalar.activation(vea[:, h * OC:(h + 1) * OC],
                                                 kvt[:, sl, D + h * OC:D + (h + 1) * OC],
                                                 AF.Copy, scale=ea[:, h:h + 1])
                            nc.vector.tensor_scalar_mul(
                                combo[:, h * 32:(h + 1) * 32], e2em[:],
                                ea[:, h:h + 1])
                        nc.vector.tensor_copy(combo[:, 64:66], ea[:])
                        nc.tensor.matmul(num_ps[:], oh[:], vea[:],
                                         start=(cc == 0), stop=False)
                        nc.tensor.matmul(extra_ps[:], oh[:], combo[:],
                                         start=(cc == 0), stop=(cc == CB - 1))

                    # ---- block epilogue
                    # fold edge-encoder value term: num += (sum ea*enc2) @ we
                    s_sb = bp.tile([128, 64], BF16, tag="ssb")
                    nc.vector.tensor_copy(s_sb[:], extra_ps[:, 0:64])
                    for h in range(2):
                        tr3 = psS.tile([32, 128], BF16, tag="s")
                        nc.tensor.transpose(tr3[:], s_sb[:, h * 32:(h + 1) * 32],
                                            ident_bf[:])
                        sT = bp.tile([32, 128], BF16, tag="sT")
                        nc.vector.tensor_copy(sT[:], tr3[:])
                        nc.tensor.matmul(num_ps[:, h * OC:(h + 1) * OC],
                                         sT[:],
                                         we_t[:, h * OC:(h + 1) * OC],
                                         start=False, stop=(h == 1))
                    dfix = bp.tile([128, 2], F32, tag="dfix")
                    nc.vector.tensor_scalar_add(dfix[:], extra_ps[:, 64:66], 1e-16)
                    rec = bp.tile([128, 2], F32, tag="rec")
                    nc.vector.reciprocal(rec[:], dfix[:])
                    att = bp.tile([128, OC], F32, tag="att")
                    nc.scalar.activation(att[:], num_ps[:, 0:OC], AF.Copy,
                                         scale=rec[:, 0:1])
                    att1 = bp.tile([128, OC], F32, tag="att1")
                    nc.scalar.activation(att1[:], num_ps[:, OC:D], AF.Copy,
                                         scale=rec[:, 1:2])
                    tsum = bp.tile([128, OC], F32, tag="tsum")
                    nc.vector.tensor_add(tsum[:], att[:], att1[:])
                    u = bp.tile([128, OC], F32, tag="u")
                    nc.vector.scalar_tensor_tensor(
                        out=u[:], in0=tsum[:], scalar=0.5, in1=skips[b][:],
                        op0=OP.mult, op1=OP.add)
                    relu_u = bp.tile([128, OC], F32, tag="relu_u")
                    nc.scalar.activation(relu_u[:], u[:], AF.Relu)
                    negu = bp.tile([128, OC], F32, tag="negu")
                    nc.vector.tensor_scalar_min(negu[:], u[:], 0.0)
                    expn = bp.tile([128, OC], F32, tag="expn")
                    nc.scalar.activation(expn[:], negu[:], AF.Exp)
                    elu = bp.tile([128, OC], F32, tag="elu")
                    nc.vector.scalar_tensor_tensor(
                        out=elu[:], in0=expn[:], scalar=-1.0, in1=relu_u[:],
                        op0=OP.add, op1=OP.add)

                    # ---- proj1
                    h_ps = psS.tile([128, HID], F32, tag="s")
                    for half in range(2):
                        tp = psS.tile([128, 128], F32, tag="s")
                        nc.tensor.transpose(tp[:], elu[:, half * 128:(half + 1) * 128],
                                            ident[:])
                        tps = bp.tile([128, 128], F32, tag="eluT")
                        nc.vector.tensor_copy(tps[:], tp[:])
                        pw = wpool.tile([128, HID], F32, tag="projw")
                        nc.sync.dma_start(pw[:],
                                          dd["proj1w"][half * 128:(half + 1) * 128, :])
                        nc.tensor.matmul(h_ps[:], tps[:], pw[:], start=(half == 0),
                                         stop=False)
                    nc.tensor.matmul(h_ps[:], ones1[:], proj1b_t[:], start=False,
                                     stop=True)
                    h_sb = bp.tile([128, HID], F32, tag="h")
                    nc.vector.tensor_copy(h_sb[:], h_ps[:])

                    # ---- pooling
                    tph = psS.tile([128, 128], F32, tag="s")
                    nc.tensor.transpose(tph[:], h_sb[:], ident[:])
                    hT = bp.tile([128, 128], F32, tag="hT")
                    nc.scalar.copy(hT[:], tph[:])
                    t_ps = psS.tile([128, HID], F32, tag="s")
                    nc.tensor.matmul(t_ps[:], hT[:], poolwq_t[:], start=True, stop=False)
                    nc.tensor.matmul(t_ps[:], ohgT_t[:, b * 128:(b + 1) * 128],
                                     ctxk_sb[:], start=False, stop=True)
                    tanh_sb = bp.tile([128, HID], F32, tag="tanh")
                    nc.scalar.activation(tanh_sb[:], t_ps[:], AF.Tanh)
                    s_t = bp.tile([128, 1], F32, tag="s")
                    scr3 = bp.tile([128, HID], F32, tag="scr3")
                    nc.vector.scalar_tensor_tensor(
                        out=scr3[:], in0=tanh_sb[:], scalar=0.0, in1=poolvB_t[:],
                        op0=OP.bypass, op1=OP.mult, accum_out=s_t[:])
                    es = bp.tile([128, 1], F32, tag="es")
                    nc.scalar.activation(es[:], s_t[:], AF.Exp,
                                         bias=nmask_t[:, b:b + 1])
                    esh = bp.tile([128, HID], F32, tag="esh")
                    nc.scalar.activation(esh[:], h_sb[:], AF.Copy, scale=es[:])
                    pp = psS.tile([128, 4], F32, tag="s")
                    nc.tensor.matmul(pp[:], esh[:], ohg_t[:, b * 4:(b + 1) * 4],
                                     start=True, stop=True)
                    nc.vector.tensor_add(pnum[:], pnum[:], pp[:])
                    ddn = psS.tile([1, 4], F32, tag="s")
                    nc.tensor.matmul(ddn[:], es[:], ohg_t[:, b * 4:(b + 1) * 4],
                                     start=True, stop=True)
                    nc.vector.tensor_add(pden[:], pden[:], ddn[:])

                # ---- pooled = num / den
                dfix2 = fin.tile([1, 4], F32, tag="dfix2" + t)
                nc.vector.tensor_scalar_add(dfix2[:], pden[:], 1e-16)
                rden = fin.tile([1, 4], F32, tag="rden" + t)
                nc.vector.reciprocal(rden[:], dfix2[:])
                bc_ps = psS.tile([128, 4], F32, tag="s")
                nc.tensor.matmul(bc_ps[:], ones1[:], rden[:], start=True, stop=True)
                bc_sb = fin.tile([128, 4], F32, tag="bc" + t)
                nc.vector.tensor_copy(bc_sb[:], bc_ps[:])
                g2 = fin.tile([128, 4], F32, tag="g2" + t)
                nc.vector.tensor_mul(g2[:], pnum[:], bc_sb[:])
                G2[t] = g2

        # ---- final head MLPs
        ones4 = fin.tile([1, 4], F32, tag="ones4")
        nc.vector.memset(ones4[:], 1.0)

        def mlp2(pre, chunks, row_off=0):
            ps = psS.tile([128, 4], F32, tag="s")
            for i, ch in enumerate(chunks):
                wt = wpool.tile([128, HID], F32, tag="mlpw")
                r0 = row_off + i * 128
                nc.sync.dma_start(wt[:], dd[pre + "w1"][r0:r0 + 128, :])
                nc.tensor.matmul(ps[:], wt[:], ch[:], start=(i == 0), stop=False)
            b1 = wtile(pre + "b1", pool=wpool)
            nc.tensor.matmul(ps[:], b1[:], ones4[:], start=False, stop=True)
            rl = fin.tile([128, 4], F32, tag="rl" + pre)
            nc.scalar.activation(rl[:], ps[:], AF.Relu)
            w2 = wtile(pre + "w2", pool=wpool)
            o = psS.tile([1, 4], F32, tag="s")
            nc.tensor.matmul(o[:], w2[:], rl[:], start=True, stop=True)
            ob = fin.tile([1, 4], F32, tag="o" + pre)
            nc.vector.tensor_copy(ob[:], o[:])
            return ob

        gf = [G2["L"], SEQ["L"][:, 0:4], G2["H"], SEQ["H"][:, 0:4],
              G2["A"], SEQ["A"][:, 0:4]]
        dg_o = mlp2("dg", gf)

        ic = []
        for a, bb in [(G2["L"], G2["A"]), (G2["H"], G2["A"])]:
            m = fin.tile([128, 4], F32, tag="im" + str(len(ic)))
            nc.vector.tensor_mul(m[:], a[:], bb[:])
            ic.append(m)
        for a, bb in [(G2["L"], G2["A"]), (G2["H"], G2["A"])]:
            sd = fin.tile([128, 4], F32, tag="isd" + str(len(ic)))
            nc.vector.tensor_sub(sd[:], a[:], bb[:])
            ab = fin.tile([128, 4], F32, tag="iab" + str(len(ic)))
            nc.scalar.activation(ab[:], sd[:], AF.Abs)
            ic.append(ab)
        inter_o = mlp2("inter", ic)

        sdl = []
        for t in types:
            sd = fin.tile([128, 4], F32, tag="sd" + t)
            nc.vector.tensor_sub(sd[:], SEQ[t][:, 0:4], SEQ[t][:, 4:8])
            sdl.append(sd)
        ddg_o = mlp2("ddg", sdl, row_off=384)

        p1 = fin.tile([1, 4], F32, tag="p1")
        nc.vector.tensor_add(p1[:], dg_o[:], inter_o[:])
        p2 = fin.tile([1, 4], F32, tag="p2")
        nc.vector.tensor_add(p2[:], p1[:], ddg_o[:])
        pf = fin.tile([1, 4], F32, tag="pf")
        nc.scalar.activation(pf[:], p2[:], AF.Identity, bias=meta["b2sum"])
        nc.sync.dma_start(pred_d[:], pf[:])

    nc.compile()
    return nc


# ----------------------------------------------------------------------------
# entry point
# ----------------------------------------------------------------------------

def kernel(**inputs):
    meta, in_maps, w = _prep(inputs)
    key = (meta["N"], meta["NB"], meta["CB"], meta["b2sum"])
    if key not in _CACHE:
        _CACHE[key] = _build(meta, w)
    nc = _CACHE[key]
    for m in in_maps:
        for k, v in w.items():
            m["w_" + k] = v
    res = run_bass_kernel_spmd(nc, in_maps, list(range(NCORES)))
    out = np.concatenate([res.results[c]["pred"][0] for c in range(NCORES)])
    return out.astype(np.float32)


# revision 16
# speedup vs baseline: 1.2783x; 1.2783x over previous
"""Trainium2 Bass kernel for nn_AffinityPredictor (GNN message passing).

Strategy: shard the 32 graphs across 8 cores (4 graphs/core, nodes split at
graph boundaries). Each core:
  - computes a full-graph combined K|V node table in bf16 (edges cross graph
    boundaries) + own-range Q/W2Q/skip tables resident in SBUF,
  - streams its edges (sorted by destination, packed into 128-node blocks
    balanced by degree) through one dma_gather of 1024-wide K|V rows per edge,
  - expands per-edge Q via a one-hot matmul (no q gather), computes edge
    attention incl. the edge-encoder contribution via host-folded wq@we.T
    weights, segment softmax via one-hot scatter matmuls accumulating
    numerator/denominator in PSUM per destination block,
  - applies skip/ELU/projection, sequence-guided attention pooling per graph,
  - runs the small head MLPs for its 4 graphs.
Host only does index preprocessing/sharding/weight folding; all activation
FLOPs run on device.
"""
import sys

if "/opt/trn_rl_repo" not in sys.path:
    sys.path.insert(0, "/opt/trn_rl_repo")

import numpy as np
import ml_dtypes
from contextlib import ExitStack

BF_NP = ml_dtypes.bfloat16

import concourse.bass as bass
import concourse.tile as tile
from concourse import bacc, mybir
from concourse.bass_utils import run_bass_kernel_spmd

F32 = mybir.dt.float32
BF16 = mybir.dt.bfloat16
I16 = mybir.dt.int16
I32 = mybir.dt.int32
AF = mybir.ActivationFunctionType
OP = mybir.AluOpType

NCORES = 8
HEADS, OC, HID = 2, 256, 128
D = HEADS * OC          # 512
GQ = 512                # edges per gather call (4 chunks of 128)
NEG = -1.0e30

_CACHE = {}


# ----------------------------------------------------------------------------
# host-side preprocessing
# ----------------------------------------------------------------------------

def _pack_blocks(own_ids, deg, NB):
    """Greedy: place nodes (desc by degree) into NB bins (<=128 nodes each),
    balancing total edges. Returns list of NB node-id lists."""
    order = np.argsort(-deg, kind="stable")
    bin_e = np.zeros(NB, np.int64)
    bin_n = np.zeros(NB, np.int64)
    blocks = [[] for _ in range(NB)]
    for i in order:
        cand = np.where(bin_n < 128)[0]
        b = cand[np.argmin(bin_e[cand])]
        blocks[b].append(int(own_ids[i]))
        bin_e[b] += int(deg[i])
        bin_n[b] += 1
    return blocks


def _wrap16(vals):
    n = len(vals)
    a = np.asarray(vals, np.int64).reshape(n // 16, 16).T.astype(np.int16)
    return np.tile(a, (8, 1))


def _wrap128(vals, dtype=np.float32):
    n = len(vals)
    return np.ascontiguousarray(np.asarray(vals, dtype).reshape(n // 128, 128).T)


def _prep(inputs):
    p = {k: np.asarray(v, np.float32) for k, v in inputs["params"].items()}
    types = ["L", "H", "A"]
    N = inputs["Lx"].shape[0]
    B = 32
    gpb = B // NCORES

    # ---- pass 1: sizes
    per = {}
    NB = 1
    for t in types:
        batch = np.asarray(inputs[t + "_batch"], np.int64)
        ei = np.asarray(inputs[t + "_edge_index"], np.int64)
        dst = ei[1]
        deg = np.bincount(dst, minlength=N)
        for c in range(NCORES):
            own = np.where((batch >= c * gpb) & (batch < (c + 1) * gpb))[0]
            per[(t, c)] = (own, deg[own])
            NB = max(NB, (len(own) + 127) // 128)

    blocks_all = {}
    CB = 1
    for t in types:
        ei = np.asarray(inputs[t + "_edge_index"], np.int64)
        dst = ei[1]
        deg_full = np.bincount(dst, minlength=N)
        for c in range(NCORES):
            own, dg = per[(t, c)]
            blocks = _pack_blocks(own, dg, NB)
            blocks_all[(t, c)] = blocks
            for bl in blocks:
                cnt = int(deg_full[bl].sum()) if bl else 0
                CB = max(CB, (cnt + 127) // 128)
    CB = ((CB + 3) // 4) * 4  # multiple of 4 so EP % GQ == 0 (NB may be odd)
    NBn = NB * 128
    EP = NB * CB * 128

    # ---- pass 2: materialize per-core arrays
    in_maps = [dict() for _ in range(NCORES)]
    for t in types:
        x = np.asarray(inputs[t + "x"], np.float32)
        ei = np.asarray(inputs[t + "_edge_index"], np.int64)
        ea = np.asarray(inputs[t + "_edge_attr"], np.float32)
        batch = np.asarray(inputs[t + "_batch"], np.int64)
        src, dst = ei[0], ei[1]

        xT_aug = np.ones((24, N), np.float32)
        xT_aug[:23] = x.T
        xT_aug = xT_aug.astype(np.float32)
        core_of_edge = batch[dst] // gpb

        for c in range(NCORES):
            blocks = blocks_all[(t, c)]
            slot = np.full(N, -1, np.int64)
            for b, bl in enumerate(blocks):
                for j, nid in enumerate(bl):
                    slot[nid] = b * 128 + j

            xT_own = np.zeros((24, NBn), np.float32)
            nmask = np.full(NBn, NEG, np.float32)
            ohg = np.zeros((NBn, 4), np.float32)
            ohgT = np.zeros((4, NBn), np.float32)
            own_ids = np.where(slot >= 0)[0]
            s_own = slot[own_ids]
            xT_own[:23, s_own] = x[own_ids].T
            xT_own[23, s_own] = 1.0
            nmask[s_own] = 0.0
            glocal = batch[own_ids] - c * gpb
            ohg[s_own, glocal] = 1.0
            ohgT[glocal, s_own] = 1.0

            emask = core_of_edge == c
            es = src[emask]
            eat = ea[emask]
            eslot = slot[dst[emask]]
            eblock = eslot // 128
            order = np.argsort(eblock, kind="stable")
            es, eat, eslot, eblock = es[order], eat[order], eslot[order], eblock[order]

            src_pad = np.zeros(EP, np.int64)
            dstl_pad = np.zeros(EP, np.float32)
            lmask_pad = np.full(EP, NEG, np.float32)
            eat_pad = np.zeros((EP, 2), np.float32)
            bstart = np.searchsorted(eblock, np.arange(NB))
            bend = np.searchsorted(eblock, np.arange(NB), side="right")
            for b in range(NB):
                s0, s1 = int(bstart[b]), int(bend[b])
                cnt = s1 - s0
                assert cnt <= CB * 128
                o = b * CB * 128
                src_pad[o:o + cnt] = es[s0:s1]
                dstl_pad[o:o + cnt] = (eslot[s0:s1] % 128).astype(np.float32)
                lmask_pad[o:o + cnt] = 0.0
                eat_pad[o:o + cnt] = eat[s0:s1]

            eattrT = np.ones((3, EP), np.float32)
            eattrT[:2] = eat_pad.T

            m = in_maps[c]
            m[t + "_xT"] = xT_aug.astype(BF_NP)
            m[t + "_xTo"] = xT_own.astype(BF_NP)
            m[t + "_srcw"] = _wrap16(src_pad)
            m[t + "_dstl"] = _wrap128(dstl_pad)
            m[t + "_dstlr"] = dstl_pad.reshape(1, EP).copy()
            m[t + "_lmask"] = _wrap128(lmask_pad)
            m[t + "_eattrT"] = eattrT.astype(BF_NP)
            m[t + "_nmask"] = np.ascontiguousarray(nmask.reshape(NB, 128).T)
            m[t + "_ohg"] = np.ascontiguousarray(
                ohg.reshape(NB, 128, 4).transpose(1, 0, 2).reshape(128, NB * 4))
            m[t + "_ohgT"] = ohgT

            sq = np.asarray(inputs[t + "s"], np.float32)[c * gpb:(c + 1) * gpb]
            wsq = np.asarray(inputs["W" + t + "s"], np.float32)[c * gpb:(c + 1) * gpb]
            seqs = np.zeros((1408, 8), np.float32)
            seqs[:1280, 0:4] = sq.T
            seqs[:1280, 4:8] = wsq.T
            seqs[1280, :] = 1.0
            m[t + "_seqs"] = seqs

    # ---- shared weights
    w = {}
    bf = BF_NP

    def aug24(wname, bname, dim):
        a = np.zeros((24, dim), np.float32)
        a[:23] = p[wname]
        a[23] = p[bname]
        return a

    wq_aug = aug24("conv_wq", "conv_bq", D)
    we = p["conv_we"]                        # [32, 512]
    # folded q@we.T per head -> [24, 64]
    wq2 = np.concatenate(
        [wq_aug[:, h * OC:(h + 1) * OC] @ we[:, h * OC:(h + 1) * OC].T
         for h in range(2)], axis=1)
    w["wqq"] = np.concatenate([wq_aug, wq2], axis=1).astype(bf)   # [24, 576]
    w["wkv"] = np.concatenate(
        [aug24("conv_wk", "conv_bk", D), aug24("conv_wv", "conv_bv", D)],
        axis=1).astype(bf)                                        # [24, 1024]
    w["wskip"] = aug24("conv_wskip", "conv_bskip", OC).astype(bf)
    eew1 = np.zeros((3, 32), np.float32)
    eew1[:2] = p["ee_w1"]
    eew1[2] = p["ee_b1"]
    w["eew1"] = eew1.astype(bf)
    w["eew2"] = p["ee_w2"].astype(bf)
    w["eeb2"] = p["ee_b2"].reshape(32, 1).astype(np.float32)
    w["we"] = we.astype(bf)
    w["proj1w"] = p["proj1_w"]
    w["proj1b"] = p["proj1_b"].reshape(1, HID).copy()
    w["poolwq"] = p["pool_wq"]
    w["poolwk"] = p["pool_wk"]
    w["poolvB"] = np.tile(p["pool_v"].reshape(1, HID), (128, 1))
    seqw = np.zeros((1408, HID), np.float32)
    seqw[:1280] = p["seq_w"]
    seqw[1280] = p["seq_b"]
    w["seqw"] = seqw
    for pre in ["dg", "ddg", "inter"]:
        w[pre + "w1"] = p[pre + "_w1"]
        w[pre + "b1"] = p[pre + "_b1"].reshape(1, HID).copy()
        w[pre + "w2"] = p[pre + "_w2"]
    b2sum = float(p["dg_b2"][0] + p["inter_b2"][0] + p["ddg_b2"][0])

    meta = dict(N=N, NB=NB, CB=CB, EP=EP, NBn=NBn, b2sum=b2sum)
    return meta, in_maps, w


# ----------------------------------------------------------------------------
# kernel builder
# ----------------------------------------------------------------------------

def _build(meta, w):
    N, NB, CB, EP, NBn = meta["N"], meta["NB"], meta["CB"], meta["EP"], meta["NBn"]
    NT = (N + 127) // 128
    types = ["L", "H", "A"]

    nc = bacc.Bacc("TRN2", target_bir_lowering=False, debug=False,
                   num_devices=NCORES, num_swdge_queues=4)

    def din(name, shape, dt=F32):
        return nc.dram_tensor(name, shape, dt, kind="ExternalInput").ap()

    dd = {}
    for t in types:
        dd[t + "_xT"] = din(t + "_xT", [24, N], BF16)
        dd[t + "_xTo"] = din(t + "_xTo", [24, NBn], BF16)
        dd[t + "_srcw"] = din(t + "_srcw", [128, EP // 16], I16)
        dd[t + "_dstl"] = din(t + "_dstl", [128, EP // 128])
        dd[t + "_dstlr"] = din(t + "_dstlr", [1, EP])
        dd[t + "_lmask"] = din(t + "_lmask", [128, EP // 128])
        dd[t + "_eattrT"] = din(t + "_eattrT", [3, EP], BF16)
        dd[t + "_nmask"] = din(t + "_nmask", [128, NB])
        dd[t + "_ohg"] = din(t + "_ohg", [128, NB * 4])
        dd[t + "_ohgT"] = din(t + "_ohgT", [4, NBn])
        dd[t + "_seqs"] = din(t + "_seqs", [1408, 8])
        dd[t + "_kvtab"] = nc.dram_tensor(t + "_kvtab", [N, 2 * D], BF16).ap()
    for k, v in w.items():
        dt = BF16 if v.dtype == BF_NP else F32
        dd[k] = din("w_" + k, list(v.shape), dt)
    pred_d = nc.dram_tensor("pred", [1, 4], F32, kind="ExternalOutput").ap()

    with tile.TileContext(nc) as tc, ExitStack() as gctx:
        const = gctx.enter_context(tc.tile_pool(name="const", bufs=1))
        wpool = gctx.enter_context(tc.tile_pool(name="wts", bufs=2))
        npool = gctx.enter_context(tc.tile_pool(name="nod", bufs=3))
        gkv = gctx.enter_context(tc.tile_pool(name="gkv", bufs=4))
        ep = gctx.enter_context(tc.tile_pool(name="ep", bufs=3))
        bp = gctx.enter_context(tc.tile_pool(name="bp", bufs=2))
        fin = gctx.enter_context(tc.tile_pool(name="fin", bufs=1))
        psA = gctx.enter_context(tc.tile_pool(name="psA", bufs=4, space="PSUM"))
        psS = gctx.enter_context(tc.tile_pool(name="psS", bufs=4, space="PSUM"))

        # ---- constants
        iota_i = const.tile([128, 128], I32)
        nc.gpsimd.iota(iota_i[:], pattern=[[1, 128]], base=0, channel_multiplier=0)
        iota_f = const.tile([128, 128], F32)
        nc.vector.tensor_copy(iota_f[:], iota_i[:])
        iotac_i = const.tile([128, 1], I32)
        nc.gpsimd.iota(iotac_i[:], pattern=[[1, 1]], base=0, channel_multiplier=1)
        iotac_f = const.tile([128, 1], F32)
        nc.vector.tensor_copy(iotac_f[:], iotac_i[:])
        ident = const.tile([128, 128], F32)
        nc.vector.tensor_scalar(out=ident[:], in0=iota_f[:], scalar1=iotac_f[:],
                                scalar2=None, op0=OP.is_equal)
        ident_bf = const.tile([128, 128], BF16)
        nc.vector.tensor_copy(ident_bf[:], ident[:])
        ones1 = const.tile([1, 128], F32)
        nc.vector.memset(ones1[:], 1.0)

        def wtile(name, pool=const):
            arr = w[name]
            dt = BF16 if arr.dtype == BF_NP else F32
            tl = pool.tile([arr.shape[0], arr.shape[1]], dt, tag="w_" + name)
            nc.sync.dma_start(tl[:], dd[name][:])
            return tl

        wqq_t = wtile("wqq"); wkv_t = wtile("wkv"); wskip_t = wtile("wskip")
        eew1_t = wtile("eew1"); eew2_t = wtile("eew2"); eeb2_t = wtile("eeb2")
        we_t = wtile("we")
        proj1b_t = wtile("proj1b")
        poolwq_t = wtile("poolwq"); poolwk_t = wtile("poolwk")
        poolvB_t = wtile("poolvB")

        G2 = {}
        SEQ = {}

        for t in types:
            with ExitStack() as tctx:
                res = tctx.enter_context(tc.tile_pool(name="res" + t, bufs=1))
                skl = tctx.enter_context(tc.tile_pool(name="skl" + t, bufs=1))

                # ---- seq encoders
                seq_ps = psS.tile([128, 8], F32, tag="s")
                for kc in range(11):
                    wt = wpool.tile([128, HID], F32, tag="seqw")
                    nc.sync.dma_start(wt[:], dd["seqw"][kc * 128:(kc + 1) * 128, :])
                    st = wpool.tile([128, 8], F32, tag="seqs")
                    nc.sync.dma_start(st[:], dd[t + "_seqs"][kc * 128:(kc + 1) * 128, :])
                    nc.tensor.matmul(seq_ps[:], wt[:], st[:], start=(kc == 0),
                                     stop=(kc == 10))
                seq_sb = fin.tile([128, 8], F32, tag="seq" + t)
                nc.vector.tensor_copy(seq_sb[:], seq_ps[:])
                SEQ[t] = seq_sb
                ctxk_ps = psS.tile([4, 128], F32, tag="s")
                nc.tensor.matmul(ctxk_ps[:], seq_sb[:, 0:4], poolwk_t[:],
                                 start=True, stop=True)
                ctxk_sb = res.tile([4, 128], F32, tag="ctxk")
                nc.vector.tensor_copy(ctxk_sb[:], ctxk_ps[:])

                # ---- resident per-type arrays
                def rload(nm, shape, dt=F32):
                    tl = res.tile(shape, dt, tag=nm)
                    nc.sync.dma_start(tl[:], dd[t + "_" + nm][:])
                    return tl

                srcw_t = rload("srcw", [128, EP // 16], I16)
                dstl_t = rload("dstl", [128, EP // 128])
                lmask_t = rload("lmask", [128, EP // 128])
                nmask_t = rload("nmask", [128, NB])
                ohg_t = rload("ohg", [128, NB * 4])
                ohgT_t = rload("ohgT", [4, NBn])

                # ---- node tables: combined K|V (bf16) -> DRAM
                for nt in range(NT):
                    cols = min(128, N - nt * 128)
                    xt = npool.tile([24, 128], BF16, tag="xt")
                    nc.sync.dma_start(xt[:, :cols],
                                      dd[t + "_xT"][:, nt * 128:nt * 128 + cols])
                    kv_sb = npool.tile([128, 2 * D], BF16, tag="kvsb")
                    for half in range(2):
                        ps = psA.tile([128, D], F32, tag="a")
                        nc.tensor.matmul(ps[:cols, :], xt[:, :cols],
                                         wkv_t[:, half * D:(half + 1) * D],
                                         start=True, stop=True)
                        if half == 0:
                            nc.vector.tensor_copy(kv_sb[:cols, 0:D], ps[:cols, :])
                        else:
                            nc.scalar.copy(kv_sb[:cols, D:2 * D], ps[:cols, :])
                    nc.sync.dma_start(
                        dd[t + "_kvtab"][nt * 128:nt * 128 + cols, :],
                        kv_sb[:cols, :])

                # ---- own-range Q|W2Q blocks (SBUF resident, bf16) + skip
                qbs = []
                skips = []
                for nt in range(NB):
                    xt = npool.tile([24, 128], BF16, tag="xt")
                    nc.sync.dma_start(xt[:], dd[t + "_xTo"][:, nt * 128:(nt + 1) * 128])
                    qb = skl.tile([128, 576], BF16, tag="qb" + str(nt))
                    ps = psA.tile([128, D], F32, tag="a")
                    nc.tensor.matmul(ps[:], xt[:], wqq_t[:, 0:D], start=True, stop=True)
                    nc.vector.tensor_copy(qb[:, 0:D], ps[:])
                    ps2 = psS.tile([128, 64], F32, tag="s")
                    nc.tensor.matmul(ps2[:], xt[:], wqq_t[:, D:576], start=True, stop=True)
                    nc.scalar.copy(qb[:, D:576], ps2[:])
                    qbs.append(qb)
                    ps3 = psS.tile([128, OC], F32, tag="s")
                    nc.tensor.matmul(ps3[:], xt[:], wskip_t[:], start=True, stop=True)
                    sk = skl.tile([128, OC], F32, tag="skip" + str(nt))
                    nc.scalar.copy(sk[:], ps3[:])
                    skips.append(sk)

                # ---- pooling accumulators
                pnum = res.tile([128, 4], F32, tag="pnum")
                nc.vector.memset(pnum[:], 0.0)
                pden = res.tile([1, 4], F32, tag="pden")
                nc.vector.memset(pden[:], 0.0)

                # ---- edge phase
                state = {}

                def gather_call(g):
                    kvt = gkv.tile([128, 4, 2 * D], BF16, tag="kv")
                    nc.gpsimd.dma_gather(kvt[:], dd[t + "_kvtab"][:],
                                         srcw_t[:, g * 32:(g + 1) * 32],
                                         num_idxs=GQ, num_idxs_reg=GQ,
                                         elem_size=2 * D, queue_num=g % 4)
                    et = ep.tile([3, GQ], BF16, tag="eat")
                    nc.sync.dma_start(et[:], dd[t + "_eattrT"][:, g * GQ:(g + 1) * GQ])
                    ps1 = psS.tile([32, GQ], F32, tag="s")
                    nc.tensor.matmul(ps1[:], eew1_t[:], et[:], start=True, stop=True)
                    e1 = ep.tile([32, GQ], BF16, tag="enc1")
                    nc.scalar.activation(e1[:], ps1[:], AF.Relu)
                    ps2 = psS.tile([32, GQ], F32, tag="s")
                    nc.tensor.matmul(ps2[:], eew2_t[:], e1[:], start=True, stop=True)
                    e2 = ep.tile([32, GQ], BF16, tag="enc2")
                    nc.scalar.activation(e2[:], ps2[:], AF.Identity, bias=eeb2_t[:])
                    drt = ep.tile([1, GQ], F32, tag="drt")
                    nc.sync.dma_start(drt[:], dd[t + "_dstlr"][:, g * GQ:(g + 1) * GQ])
                    dB = psS.tile([128, GQ], F32, tag="s")
                    nc.tensor.matmul(dB[:], ones1[:], drt[:], start=True, stop=True)
                    ohT4 = ep.tile([128, 4, 128], BF16, tag="ohT4", bufs=3)
                    nc.vector.tensor_scalar(
                        out=ohT4[:], in0=dB.rearrange("p (c e) -> p c e", c=4)[:],
                        scalar1=iotac_f[:], scalar2=None, op0=OP.is_equal)
                    oh4 = ep.tile([128, 4, 128], BF16, tag="oh4", bufs=3)
                    dcs = dstl_t[:, g * 4:(g + 1) * 4]
                    in0 = bass.AP(tensor=dcs.tensor, offset=dcs.offset,
                                  ap=[dcs.ap[0], [dcs.ap[1][0], 4], [0, 128]])
                    in1 = bass.AP(tensor=iota_f.tensor, offset=iota_f.offset,
                                  ap=[iota_f.ap[0], [0, 4], [1, 128]])
                    nc.vector.tensor_tensor(out=oh4[:], in0=in0, in1=in1, op=OP.is_equal)
                    state["kv"], state["e2"] = kvt, e2
                    state["oh4"], state["ohT4"] = oh4, ohT4

                for b in range(NB):
                    num_ps = psA.tile([128, D], F32, tag="a")
                    extra_ps = psS.tile([128, 66], F32, tag="s")
                    for cc in range(CB):
                        gidx = b * CB + cc
                        g, sl = gidx // 4, gidx % 4
                        if sl == 0:
                            gather_call(g)
                        kvt, e2 = state["kv"], state["e2"]
                        e2c = e2[:, sl * 128:(sl + 1) * 128]
                        tr1 = psS.tile([128, 32], BF16, tag="s")
                        nc.tensor.transpose(tr1[:], e2c[:], ident_bf[:32, :32])
                        e2em = ep.tile([128, 32], BF16, tag="e2em")
                        nc.vector.tensor_copy(e2em[:], tr1[:])
                        oh = state["oh4"][:, sl, :]
                        ohT = state["ohT4"][:, sl, :]
                        qexa = psA.tile([128, D], F32, tag="a")
                        nc.tensor.matmul(qexa[:], ohT[:], qbs[b][:, 0:D], start=True, stop=True)
                        qexb = psS.tile([128, 64], F32, tag="s")
                        nc.tensor.matmul(qexb[:], ohT[:], qbs[b][:, D:576], start=True, stop=True)
                        aqk = ep.tile([128, 2], F32, tag="aqk")
                        aee = ep.tile([128, 2], F32, tag="aee")
                        scr = ep.tile([128, OC], F32, tag="scr")
                        scr2 = ep.tile([128, 32], F32, tag="scr32")
                        for h in range(2):
                            nc.vector.scalar_tensor_tensor(
                                out=scr[:], in0=kvt[:, sl, h * OC:(h + 1) * OC],
                                scalar=0.0625, in1=qexa[:, h * OC:(h + 1) * OC],
                                op0=OP.mult, op1=OP.mult,
                                accum_out=aqk[:, h:h + 1])
                            nc.vector.scalar_tensor_tensor(
                                out=scr2[:], in0=e2em[:],
                                scalar=0.0625, in1=qexb[:, h * 32:(h + 1) * 32],
                                op0=OP.mult, op1=OP.mult,
                                accum_out=aee[:, h:h + 1])
                        alpha = ep.tile([128, 2], F32, tag="alpha")
                        nc.vector.scalar_tensor_tensor(
                            out=alpha[:], in0=aqk[:],
                            scalar=lmask_t[:, gidx:gidx + 1], in1=aee[:],
                            op0=OP.add, op1=OP.add)
                        ea = ep.tile([128, 2], F32, tag="ea")
                        nc.scalar.activation(ea[:], alpha[:], AF.Exp)
                        vea = ep.tile([128, D], BF16, tag="vea")
                        combo = ep.tile([128, 66], BF16, tag="combo")
                        for h in range(2):
                            nc.scalar.activation(vea[:, h * OC:(h + 1) * OC],
                                                 kvt[:, sl, D + h * OC:D + (h + 1) * OC],
                                                 AF.Copy, scale=ea[:, h:h + 1])
                            nc.vector.tensor_scalar_mul(
                                combo[:, h * 32:(h + 1) * 32], e2em[:],
                                ea[:, h:h + 1])
                        nc.vector.tensor_copy(combo[:, 64:66], ea[:])
                        nc.tensor.matmul(num_ps[:], oh[:], vea[:],
                                         start=(cc == 0), stop=False)
                        nc.tensor.matmul(extra_ps[:], oh[:], combo[:],
                                         start=(cc == 0), stop=(cc == CB - 1))

                    # ---- block epilogue
                    # fold edge-encoder value term: num += (sum ea*enc2) @ we
                    s_sb = bp.tile([128, 64], BF16, tag="ssb")
                    nc.vector.tensor_copy(s_sb[:], extra_ps[:, 0:64])
                    for h in range(2):
                        tr3 = psS.tile([32, 128], BF16, tag="s")
                        nc.tensor.transpose(tr3[:], s_sb[:, h * 32:(h + 1) * 32],
                                            ident_bf[:])
                        sT = bp.tile([32, 128], BF16, tag="sT")
                        nc.vector.tensor_copy(sT[:], tr3[:])
                        nc.tensor.matmul(num_ps[:, h * OC:(h + 1) * OC],
                                         sT[:],
                                         we_t[:, h * OC:(h + 1) * OC],
                                         start=False, stop=(h == 1))
                    dfix = bp.tile([128, 2], F32, tag="dfix")
                    nc.vector.tensor_scalar_add(dfix[:], extra_ps[:, 64:66], 1e-16)
                    rec = bp.tile([128, 2], F32, tag="rec")
                    nc.vector.reciprocal(rec[:], dfix[:])
                    att = bp.tile([128, OC], F32, tag="att")
                    nc.scalar.activation(att[:], num_ps[:, 0:OC], AF.Copy,
                                         scale=rec[:, 0:1])
                    att1 = bp.tile([128, OC], F32, tag="att1")
                    nc.scalar.activation(att1[:], num_ps[:, OC:D], AF.Copy,
                                         scale=rec[:, 1:2])
                    tsum = bp.tile([128, OC], F32, tag="tsum")
                    nc.vector.tensor_add(tsum[:], att[:], att1[:])
                    u = bp.tile([128, OC], F32, tag="u")
                    nc.vector.scalar_tensor_tensor(
                        out=u[:], in0=tsum[:], scalar=0.5, in1=skips[b][:],
                        op0=OP.mult, op1=OP.add)
                    relu_u = bp.tile([128, OC], F32, tag="relu_u")
                    nc.scalar.activation(relu_u[:], u[:], AF.Relu)
                    negu = bp.tile([128, OC], F32, tag="negu")
                    nc.vector.tensor_scalar_min(negu[:], u[:], 0.0)
                    expn = bp.tile([128, OC], F32, tag="expn")
                    nc.scalar.activation(expn[:], negu[:], AF.Exp)
                    elu = bp.tile([128, OC], F32, tag="elu")
                    nc.vector.scalar_tensor_tensor(
                        out=elu[:], in0=expn[:], scalar=-1.0, in1=relu_u[:],
                        op0=OP.add, op1=OP.add)

                    # ---- proj1
                    h_ps = psS.tile([128, HID], F32, tag="s")
                    for half in range(2):
                        tp = psS.tile([128, 128], F32, tag="s")
                        nc.tensor.transpose(tp[:], elu[:, half * 128:(half + 1) * 128],
                                            ident[:])
                        tps = bp.tile([128, 128], F32, tag="eluT")
                        nc.vector.tensor_copy(tps[:], tp[:])
                        pw = wpool.tile([128, HID], F32, tag="projw")
                        nc.sync.dma_start(pw[:],
                                          dd["proj1w"][half * 128:(half + 1) * 128, :])
                        nc.tensor.matmul(h_ps[:], tps[:], pw[:], start=(half == 0),
                                         stop=False)
                    nc.tensor.matmul(h_ps[:], ones1[:], proj1b_t[:], start=False,
                                     stop=True)
                    h_sb = bp.tile([128, HID], F32, tag="h")
                    nc.vector.tensor_copy(h_sb[:], h_ps[:])

                    # ---- pooling
                    tph = psS.tile([128, 128], F32, tag="s")
                    nc.tensor.transpose(tph[:], h_sb[:], ident[:])
                    hT = bp.tile([128, 128], F32, tag="hT")
                    nc.scalar.copy(hT[:], tph[:])
                    t_ps = psS.tile([128, HID], F32, tag="s")
                    nc.tensor.matmul(t_ps[:], hT[:], poolwq_t[:], start=True, stop=False)
                    nc.tensor.matmul(t_ps[:], ohgT_t[:, b * 128:(b + 1) * 128],
                                     ctxk_sb[:], start=False, stop=True)
                    tanh_sb = bp.tile([128, HID], F32, tag="tanh")
                    nc.scalar.activation(tanh_sb[:], t_ps[:], AF.Tanh)
                    s_t = bp.tile([128, 1], F32, tag="s")
                    scr3 = bp.tile([128, HID], F32, tag="scr3")
                    nc.vector.scalar_tensor_tensor(
                        out=scr3[:], in0=tanh_sb[:], scalar=0.0, in1=poolvB_t[:],
                        op0=OP.bypass, op1=OP.mult, accum_out=s_t[:])
                    es = bp.tile([128, 1], F32, tag="es")
                    nc.scalar.activation(es[:], s_t[:], AF.Exp,
                                         bias=nmask_t[:, b:b + 1])
                    esh = bp.tile([128, HID], F32, tag="esh")
                    nc.scalar.activation(esh[:], h_sb[:], AF.Copy, scale=es[:])
                    pp = psS.tile([128, 4], F32, tag="s")
                    nc.tensor.matmul(pp[:], esh[:], ohg_t[:, b * 4:(b + 1) * 4],
                                     start=True, stop=True)
                    nc.vector.tensor_add(pnum[:], pnum[:], pp[:])
                    ddn = psS.tile([1, 4], F32, tag="s")
                    nc.tensor.matmul(ddn[:], es[:], ohg_t[:, b * 4:(b + 1) * 4],
                                     start=True, stop=True)
                    nc.vector.tensor_add(pden[:], pden[:], ddn[:])

                # ---- pooled = num / den
                dfix2 = fin.tile([1, 4], F32, tag="dfix2" + t)
                nc.vector.tensor_scalar_add(dfix2[:], pden[:], 1e-16)
                rden = fin.tile([1, 4], F32, tag="rden" + t)
                nc.vector.reciprocal(rden[:], dfix2[:])
                bc_ps = psS.tile([128, 4], F32, tag="s")
                nc.tensor.matmul(bc_ps[:], ones1[:], rden[:], start=True, stop=True)
                bc_sb = fin.tile([128, 4], F32, tag="bc" + t)
                nc.vector.tensor_copy(bc_sb[:], bc_ps[:])
                g2 = fin.tile([128, 4], F32, tag="g2" + t)
                nc.vector.tensor_mul(g2[:], pnum[:], bc_sb[:])
                G2[t] = g2

        # ---- final head MLPs
        ones4 = fin.tile([1, 4], F32, tag="ones4")
        nc.vector.memset(ones4[:], 1.0)

        def mlp2(pre, chunks, row_off=0):
            ps = psS.tile([128, 4], F32, tag="s")
            for i, ch in enumerate(chunks):
                wt = wpool.tile([128, HID], F32, tag="mlpw")
                r0 = row_off + i * 128
                nc.sync.dma_start(wt[:], dd[pre + "w1"][r0:r0 + 128, :])
                nc.tensor.matmul(ps[:], wt[:], ch[:], start=(i == 0), stop=False)
            b1 = wtile(pre + "b1", pool=wpool)
            nc.tensor.matmul(ps[:], b1[:], ones4[:], start=False, stop=True)
            rl = fin.tile([128, 4], F32, tag="rl" + pre)
            nc.scalar.activation(rl[:], ps[:], AF.Relu)
            w2 = wtile(pre + "w2", pool=wpool)
            o = psS.tile([1, 4], F32, tag="s")
            nc.tensor.matmul(o[:], w2[:], rl[:], start=True, stop=True)
            ob = fin.tile([1, 4], F32, tag="o" + pre)
            nc.vector.tensor_copy(ob[:], o[:])
            return ob

        gf = [G2["L"], SEQ["L"][:, 0:4], G2["H"], SEQ["H"][:, 0:4],
              G2["A"], SEQ["A"][:, 0:4]]
        dg_o = mlp2("dg", gf)

        ic = []
        for a, bb in [(G2["L"], G2["A"]), (G2["H"], G2["A"])]:
            m = fin.tile([128, 4], F32, tag="im" + str(len(ic)))
            nc.vector.tensor_mul(m[:], a[:], bb[:])
            ic.append(m)
        for a, bb in [(G2["L"], G2["A"]), (G2["H"], G2["A"])]:
            sd = fin.tile([128, 4], F32, tag="isd" + str(len(ic)))
            nc.vector.tensor_sub(sd[:], a[:], bb[:])
            ab = fin.tile([128, 4], F32, tag="iab" + str(len(ic)))
            nc.scalar.activation(ab[:], sd[:], AF.Abs)
            ic.append(ab)
        inter_o = mlp2("inter", ic)

        sdl = []
        for t in types:
            sd = fin.tile([128, 4], F32, tag="sd" + t)
            nc.vector.tensor_sub(sd[:], SEQ[t][:, 0:4], SEQ[t][:, 4:8])
            sdl.append(sd)
        ddg_o = mlp2("ddg", sdl, row_off=384)

        p1 = fin.tile([1, 4], F32, tag="p1")
        nc.vector.tensor_add(p1[:], dg_o[:], inter_o[:])
        p2 = fin.tile([1, 4], F32, tag="p2")
        nc.vector.tensor_add(p2[:], p1[:], ddg_o[:])
        pf = fin.tile([1, 4], F32, tag="pf")
        nc.scalar.activation(pf[:], p2[:], AF.Identity, bias=meta["b2sum"])
        nc.sync.dma_start(pred_d[:], pf[:])

    nc.compile()
    return nc


# ----------------------------------------------------------------------------
# entry point
# ----------------------------------------------------------------------------

def kernel(**inputs):
    meta, in_maps, w = _prep(inputs)
    key = (meta["N"], meta["NB"], meta["CB"], meta["b2sum"])
    if key not in _CACHE:
        _CACHE[key] = _build(meta, w)
    nc = _CACHE[key]
    for m in in_maps:
        for k, v in w.items():
            m["w_" + k] = v
    res = run_bass_kernel_spmd(nc, in_maps, list(range(NCORES)))
    out = np.concatenate([res.results[c]["pred"][0] for c in range(NCORES)])
    return out.astype(np.float32)


# revision 21
# speedup vs baseline: 1.4720x; 1.1516x over previous
"""Trainium2 Bass kernel for nn_AffinityPredictor (GNN message passing).

Strategy: shard the 32 graphs across 8 cores (4 graphs/core, nodes split at
graph boundaries). Each core:
  - computes a full-graph combined K|V node table in bf16 (edges cross graph
    boundaries) + own-range Q/W2Q/skip tables resident in SBUF,
  - streams its edges (sorted by destination, packed into 128-node blocks
    balanced by degree) through one dma_gather of 1024-wide K|V rows per edge,
  - expands per-edge Q via a one-hot matmul (no q gather), computes edge
    attention incl. the edge-encoder contribution via host-folded wq@we.T
    weights, segment softmax via one-hot scatter matmuls accumulating
    numerator/denominator in PSUM per destination block,
  - applies skip/ELU/projection, sequence-guided attention pooling per graph,
  - runs the small head MLPs for its 4 graphs.
Host only does index preprocessing/sharding/weight folding; all activation
FLOPs run on device.
"""
import sys

if "/opt/trn_rl_repo" not in sys.path:
    sys.path.insert(0, "/opt/trn_rl_repo")

import numpy as np
import ml_dtypes
from contextlib import ExitStack

BF_NP = ml_dtypes.bfloat16

import concourse.bass as bass
import concourse.tile as tile
from concourse.tile import add_dep_helper
from concourse import bacc, mybir
from concourse.bass_utils import run_bass_kernel_spmd

F32 = mybir.dt.float32
BF16 = mybir.dt.bfloat16
I16 = mybir.dt.int16
I32 = mybir.dt.int32
AF = mybir.ActivationFunctionType
OP = mybir.AluOpType

NCORES = 8
HEADS, OC, HID = 2, 256, 128
D = HEADS * OC          # 512
GQ = 512                # edges per gather call (4 chunks of 128)
NEG = -1.0e30

_CACHE = {}


# ----------------------------------------------------------------------------
# host-side preprocessing
# ----------------------------------------------------------------------------

def _pack_blocks(own_ids, deg, NB):
    """Greedy: place nodes (desc by degree) into NB bins (<=128 nodes each),
    balancing total edges. Returns list of NB node-id lists."""
    order = np.argsort(-deg, kind="stable")
    bin_e = np.zeros(NB, np.int64)
    bin_n = np.zeros(NB, np.int64)
    blocks = [[] for _ in range(NB)]
    for i in order:
        cand = np.where(bin_n < 128)[0]
        b = cand[np.argmin(bin_e[cand])]
        blocks[b].append(int(own_ids[i]))
        bin_e[b] += int(deg[i])
        bin_n[b] += 1
    return blocks


def _wrap16(vals):
    n = len(vals)
    a = np.asarray(vals, np.int64).reshape(n // 16, 16).T.astype(np.int16)
    return np.tile(a, (8, 1))


def _wrap128(vals, dtype=np.float32):
    n = len(vals)
    return np.ascontiguousarray(np.asarray(vals, dtype).reshape(n // 128, 128).T)


def _prep(inputs):
    p = {k: np.asarray(v, np.float32) for k, v in inputs["params"].items()}
    types = ["L", "H", "A"]
    N = inputs["Lx"].shape[0]
    B = 32
    gpb = B // NCORES

    # ---- pass 1: sizes
    per = {}
    NB = 1
    for t in types:
        batch = np.asarray(inputs[t + "_batch"], np.int64)
        ei = np.asarray(inputs[t + "_edge_index"], np.int64)
        dst = ei[1]
        deg = np.bincount(dst, minlength=N)
        for c in range(NCORES):
            own = np.where((batch >= c * gpb) & (batch < (c + 1) * gpb))[0]
            per[(t, c)] = (own, deg[own])
            NB = max(NB, (len(own) + 127) // 128)

    blocks_all = {}
    CB = 1
    for t in types:
        ei = np.asarray(inputs[t + "_edge_index"], np.int64)
        dst = ei[1]
        deg_full = np.bincount(dst, minlength=N)
        for c in range(NCORES):
            own, dg = per[(t, c)]
            blocks = _pack_blocks(own, dg, NB)
            blocks_all[(t, c)] = blocks
            for bl in blocks:
                cnt = int(deg_full[bl].sum()) if bl else 0
                CB = max(CB, (cnt + 127) // 128)
    CB = ((CB + 3) // 4) * 4  # multiple of 4 so EP % GQ == 0 (NB may be odd)
    NBn = NB * 128
    EP = NB * CB * 128

    # ---- pass 2: materialize per-core arrays
    in_maps = [dict() for _ in range(NCORES)]
    for t in types:
        x = np.asarray(inputs[t + "x"], np.float32)
        ei = np.asarray(inputs[t + "_edge_index"], np.int64)
        ea = np.asarray(inputs[t + "_edge_attr"], np.float32)
        batch = np.asarray(inputs[t + "_batch"], np.int64)
        src, dst = ei[0], ei[1]

        xT_aug = np.ones((24, N), np.float32)
        xT_aug[:23] = x.T
        xT_aug = xT_aug.astype(np.float32)
        core_of_edge = batch[dst] // gpb

        for c in range(NCORES):
            blocks = blocks_all[(t, c)]
            slot = np.full(N, -1, np.int64)
            for b, bl in enumerate(blocks):
                for j, nid in enumerate(bl):
                    slot[nid] = b * 128 + j

            xT_own = np.zeros((24, NBn), np.float32)
            nmask = np.full(NBn, NEG, np.float32)
            ohg = np.zeros((NBn, 4), np.float32)
            ohgT = np.zeros((4, NBn), np.float32)
            own_ids = np.where(slot >= 0)[0]
            s_own = slot[own_ids]
            xT_own[:23, s_own] = x[own_ids].T
            xT_own[23, s_own] = 1.0
            nmask[s_own] = 0.0
            glocal = batch[own_ids] - c * gpb
            ohg[s_own, glocal] = 1.0
            ohgT[glocal, s_own] = 1.0

            emask = core_of_edge == c
            es = src[emask]
            eat = ea[emask]
            eslot = slot[dst[emask]]
            eblock = eslot // 128
            order = np.argsort(eblock, kind="stable")
            es, eat, eslot, eblock = es[order], eat[order], eslot[order], eblock[order]

            src_pad = np.zeros(EP, np.int64)
            dstq_pad = np.zeros(EP, np.int64)
            dstl_pad = np.zeros(EP, np.float32)
            lmask_pad = np.full(EP, NEG, np.float32)
            eat_pad = np.zeros((EP, 2), np.float32)
            bstart = np.searchsorted(eblock, np.arange(NB))
            bend = np.searchsorted(eblock, np.arange(NB), side="right")
            for b in range(NB):
                s0, s1 = int(bstart[b]), int(bend[b])
                cnt = s1 - s0
                assert cnt <= CB * 128
                o = b * CB * 128
                src_pad[o:o + cnt] = es[s0:s1]
                dstq_pad[o:o + cnt] = eslot[s0:s1]
                dstl_pad[o:o + cnt] = (eslot[s0:s1] % 128).astype(np.float32)
                lmask_pad[o:o + cnt] = 0.0
                eat_pad[o:o + cnt] = eat[s0:s1]

            eattrT = np.ones((3, EP), np.float32)
            eattrT[:2] = eat_pad.T

            m = in_maps[c]
            m[t + "_xT"] = xT_aug.astype(BF_NP)
            m[t + "_xTo"] = xT_own.astype(BF_NP)
            m[t + "_srcw"] = _wrap16(src_pad)
            m[t + "_dstqw"] = _wrap16(dstq_pad)
            m[t + "_dstl"] = _wrap128(dstl_pad)
            m[t + "_dstlr"] = dstl_pad.reshape(1, EP).copy()
            m[t + "_lmask"] = _wrap128(lmask_pad)
            m[t + "_eattrT"] = eattrT.astype(BF_NP)
            m[t + "_nmask"] = np.ascontiguousarray(nmask.reshape(NB, 128).T)
            m[t + "_ohg"] = np.ascontiguousarray(
                ohg.reshape(NB, 128, 4).transpose(1, 0, 2).reshape(128, NB * 4))
            m[t + "_ohgT"] = ohgT

            sq = np.asarray(inputs[t + "s"], np.float32)[c * gpb:(c + 1) * gpb]
            wsq = np.asarray(inputs["W" + t + "s"], np.float32)[c * gpb:(c + 1) * gpb]
            seqs = np.zeros((1408, 8), np.float32)
            seqs[:1280, 0:4] = sq.T
            seqs[:1280, 4:8] = wsq.T
            seqs[1280, :] = 1.0
            m[t + "_seqs"] = seqs

    # ---- shared weights
    w = {}
    bf = BF_NP

    def aug24(wname, bname, dim):
        a = np.zeros((24, dim), np.float32)
        a[:23] = p[wname]
        a[23] = p[bname]
        return a

    wq_aug = aug24("conv_wq", "conv_bq", D)
    we = p["conv_we"]                        # [32, 512]
    # folded q@we.T per head -> [24, 64]
    wq2 = np.concatenate(
        [wq_aug[:, h * OC:(h + 1) * OC] @ we[:, h * OC:(h + 1) * OC].T
         for h in range(2)], axis=1)
    w["wqq"] = np.concatenate([wq_aug, wq2], axis=1).astype(bf)   # [24, 576]
    w["wkv"] = np.concatenate(
        [aug24("conv_wk", "conv_bk", D), aug24("conv_wv", "conv_bv", D)],
        axis=1).astype(bf)                                        # [24, 1024]
    w["wskip"] = aug24("conv_wskip", "conv_bskip", OC).astype(bf)
    eew1 = np.zeros((3, 32), np.float32)
    eew1[:2] = p["ee_w1"]
    eew1[2] = p["ee_b1"]
    w["eew1"] = eew1.astype(bf)
    w["eew2"] = p["ee_w2"].astype(bf)
    w["eeb2"] = p["ee_b2"].reshape(32, 1).astype(np.float32)
    w["we"] = we.astype(bf)
    w["proj1w"] = p["proj1_w"]
    w["proj1b"] = p["proj1_b"].reshape(1, HID).copy()
    w["poolwq"] = p["pool_wq"]
    w["poolwk"] = p["pool_wk"]
    w["poolvB"] = np.tile(p["pool_v"].reshape(1, HID), (128, 1))
    seqw = np.zeros((1408, HID), np.float32)
    seqw[:1280] = p["seq_w"]
    seqw[1280] = p["seq_b"]
    w["seqw"] = seqw
    for pre in ["dg", "ddg", "inter"]:
        w[pre + "w1"] = p[pre + "_w1"]
        w[pre + "b1"] = p[pre + "_b1"].reshape(1, HID).copy()
        w[pre + "w2"] = p[pre + "_w2"]
    b2sum = float(p["dg_b2"][0] + p["inter_b2"][0] + p["ddg_b2"][0])

    meta = dict(N=N, NB=NB, CB=CB, EP=EP, NBn=NBn, b2sum=b2sum)
    return meta, in_maps, w


# ----------------------------------------------------------------------------
# kernel builder
# ----------------------------------------------------------------------------

def _build(meta, w):
    N, NB, CB, EP, NBn = meta["N"], meta["NB"], meta["CB"], meta["EP"], meta["NBn"]
    NT = (N + 127) // 128
    types = ["L", "H", "A"]

    nc = bacc.Bacc("TRN2", target_bir_lowering=False, debug=False,
                   num_devices=NCORES, num_swdge_queues=4)

    def din(name, shape, dt=F32):
        return nc.dram_tensor(name, shape, dt, kind="ExternalInput").ap()

    dd = {}
    for t in types:
        dd[t + "_xT"] = din(t + "_xT", [24, N], BF16)
        dd[t + "_xTo"] = din(t + "_xTo", [24, NBn], BF16)
        dd[t + "_srcw"] = din(t + "_srcw", [128, EP // 16], I16)
        dd[t + "_dstqw"] = din(t + "_dstqw", [128, EP // 16], I16)
        dd[t + "_dstl"] = din(t + "_dstl", [128, EP // 128])
        dd[t + "_dstlr"] = din(t + "_dstlr", [1, EP])
        dd[t + "_lmask"] = din(t + "_lmask", [128, EP // 128])
        dd[t + "_eattrT"] = din(t + "_eattrT", [3, EP], BF16)
        dd[t + "_nmask"] = din(t + "_nmask", [128, NB])
        dd[t + "_ohg"] = din(t + "_ohg", [128, NB * 4])
        dd[t + "_ohgT"] = din(t + "_ohgT", [4, NBn])
        dd[t + "_seqs"] = din(t + "_seqs", [1408, 8])
        dd[t + "_kvtab"] = nc.dram_tensor(t + "_kvtab", [N, 2 * D], BF16).ap()
        dd[t + "_qtab"] = nc.dram_tensor(t + "_qtab", [NBn, 640], BF16).ap()
    for k, v in w.items():
        dt = BF16 if v.dtype == BF_NP else F32
        dd[k] = din("w_" + k, list(v.shape), dt)
    pred_d = nc.dram_tensor("pred", [1, 4], F32, kind="ExternalOutput").ap()

    with tile.TileContext(nc) as tc, ExitStack() as gctx:
        const = gctx.enter_context(tc.tile_pool(name="const", bufs=1))
        wpool = gctx.enter_context(tc.tile_pool(name="wts", bufs=2))
        npool = gctx.enter_context(tc.tile_pool(name="nod", bufs=3))
        gkv = gctx.enter_context(tc.tile_pool(name="gkv", bufs=4))
        gq = gctx.enter_context(tc.tile_pool(name="gq", bufs=4))
        ep = gctx.enter_context(tc.tile_pool(name="ep", bufs=3))
        bp = gctx.enter_context(tc.tile_pool(name="bp", bufs=2))
        fin = gctx.enter_context(tc.tile_pool(name="fin", bufs=1))
        psA = gctx.enter_context(tc.tile_pool(name="psA", bufs=4, space="PSUM"))
        psS = gctx.enter_context(tc.tile_pool(name="psS", bufs=4, space="PSUM"))

        # ---- constants
        iota_i = const.tile([128, 128], I32)
        nc.gpsimd.iota(iota_i[:], pattern=[[1, 128]], base=0, channel_multiplier=0)
        iota_f = const.tile([128, 128], F32)
        nc.vector.tensor_copy(iota_f[:], iota_i[:])
        iotac_i = const.tile([128, 1], I32)
        nc.gpsimd.iota(iotac_i[:], pattern=[[1, 1]], base=0, channel_multiplier=1)
        iotac_f = const.tile([128, 1], F32)
        nc.vector.tensor_copy(iotac_f[:], iotac_i[:])
        ident = const.tile([128, 128], F32)
        nc.vector.tensor_scalar(out=ident[:], in0=iota_f[:], scalar1=iotac_f[:],
                                scalar2=None, op0=OP.is_equal)
        ident_bf = const.tile([128, 128], BF16)
        nc.vector.tensor_copy(ident_bf[:], ident[:])
        ones1 = const.tile([1, 128], F32)
        nc.vector.memset(ones1[:], 1.0)

        def wtile(name, pool=const):
            arr = w[name]
            dt = BF16 if arr.dtype == BF_NP else F32
            tl = pool.tile([arr.shape[0], arr.shape[1]], dt, tag="w_" + name)
            nc.sync.dma_start(tl[:], dd[name][:])
            return tl

        wqq_t = wtile("wqq"); wkv_t = wtile("wkv"); wskip_t = wtile("wskip")
        eew1_t = wtile("eew1"); eew2_t = wtile("eew2"); eeb2_t = wtile("eeb2")
        we_t = wtile("we")
        proj1b_t = wtile("proj1b")
        poolwq_t = wtile("poolwq"); poolwk_t = wtile("poolwk")
        poolvB_t = wtile("poolvB")

        G2 = {}
        SEQ = {}

        for t in types:
            with ExitStack() as tctx:
                res = tctx.enter_context(tc.tile_pool(name="res" + t, bufs=1))
                skl = tctx.enter_context(tc.tile_pool(name="skl" + t, bufs=1))

                # ---- seq encoders
                seq_ps = psS.tile([128, 8], F32, tag="s")
                for kc in range(11):
                    wt = wpool.tile([128, HID], F32, tag="seqw")
                    nc.sync.dma_start(wt[:], dd["seqw"][kc * 128:(kc + 1) * 128, :])
                    st = wpool.tile([128, 8], F32, tag="seqs")
                    nc.sync.dma_start(st[:], dd[t + "_seqs"][kc * 128:(kc + 1) * 128, :])
                    nc.tensor.matmul(seq_ps[:], wt[:], st[:], start=(kc == 0),
                                     stop=(kc == 10))
                seq_sb = fin.tile([128, 8], F32, tag="seq" + t)
                nc.vector.tensor_copy(seq_sb[:], seq_ps[:])
                SEQ[t] = seq_sb
                ctxk_ps = psS.tile([4, 128], F32, tag="s")
                nc.tensor.matmul(ctxk_ps[:], seq_sb[:, 0:4], poolwk_t[:],
                                 start=True, stop=True)
                ctxk_sb = res.tile([4, 128], F32, tag="ctxk")
                nc.vector.tensor_copy(ctxk_sb[:], ctxk_ps[:])

                # ---- resident per-type arrays
                def rload(nm, shape, dt=F32):
                    tl = res.tile(shape, dt, tag=nm)
                    nc.sync.dma_start(tl[:], dd[t + "_" + nm][:])
                    return tl

                srcw_t = rload("srcw", [128, EP // 16], I16)
                dstqw_t = rload("dstqw", [128, EP // 16], I16)
                dstl_t = rload("dstl", [128, EP // 128])
                lmask_t = rload("lmask", [128, EP // 128])
                nmask_t = rload("nmask", [128, NB])
                ohg_t = rload("ohg", [128, NB * 4])
                ohgT_t = rload("ohgT", [4, NBn])

                # ---- node tables: combined K|V (bf16) -> DRAM
                for nt in range(NT):
                    cols = min(128, N - nt * 128)
                    xt = npool.tile([24, 128], BF16, tag="xt")
                    nc.sync.dma_start(xt[:, :cols],
                                      dd[t + "_xT"][:, nt * 128:nt * 128 + cols])
                    kv_sb = npool.tile([128, 2 * D], BF16, tag="kvsb")
                    for half in range(2):
                        ps = psA.tile([128, D], F32, tag="a")
                        nc.tensor.matmul(ps[:cols, :], xt[:, :cols],
                                         wkv_t[:, half * D:(half + 1) * D],
                                         start=True, stop=True)
                        if half == 0:
                            nc.vector.tensor_copy(kv_sb[:cols, 0:D], ps[:cols, :])
                        else:
                            nc.scalar.copy(kv_sb[:cols, D:2 * D], ps[:cols, :])
                    nc.sync.dma_start(
                        dd[t + "_kvtab"][nt * 128:nt * 128 + cols, :],
                        kv_sb[:cols, :])

                # ---- own-range Q|W2Q table (DRAM, bf16) + skip
                skips = []
                for nt in range(NB):
                    xt = npool.tile([24, 128], BF16, tag="xt")
                    nc.sync.dma_start(xt[:], dd[t + "_xTo"][:, nt * 128:(nt + 1) * 128])
                    qsb = npool.tile([128, 640], BF16, tag="qsb")
                    nc.vector.memset(qsb[:, 576:640], 0.0)
                    ps = psA.tile([128, D], F32, tag="a")
                    nc.tensor.matmul(ps[:], xt[:], wqq_t[:, 0:D], start=True, stop=True)
                    nc.vector.tensor_copy(qsb[:, 0:D], ps[:])
                    ps2 = psS.tile([128, 64], F32, tag="s")
                    nc.tensor.matmul(ps2[:], xt[:], wqq_t[:, D:576], start=True, stop=True)
                    nc.scalar.copy(qsb[:, D:576], ps2[:])
                    nc.sync.dma_start(dd[t + "_qtab"][nt * 128:(nt + 1) * 128, :], qsb[:])
                    ps3 = psS.tile([128, OC], F32, tag="s")
                    nc.tensor.matmul(ps3[:], xt[:], wskip_t[:], start=True, stop=True)
                    sk = skl.tile([128, OC], F32, tag="skip" + str(nt))
                    nc.scalar.copy(sk[:], ps3[:])
                    skips.append(sk)
                    skips.append(sk)

                # ---- pooling accumulators
                pnum = res.tile([128, 4], F32, tag="pnum")
                nc.vector.memset(pnum[:], 0.0)
                pden = res.tile([1, 4], F32, tag="pden")
                nc.vector.memset(pden[:], 0.0)

                # ---- edge phase
                state = {}

                def gather_call(g):
                    kvt = gkv.tile([128, 4, 2 * D], BF16, tag="kv")
                    qt = gq.tile([128, 4, 640], BF16, tag="qg")
                    i1 = nc.gpsimd.dma_gather(kvt[:], dd[t + "_kvtab"][:],
                                              srcw_t[:, g * 32:(g + 1) * 32],
                                              num_idxs=GQ, num_idxs_reg=GQ,
                                              elem_size=2 * D, queue_num=(2 * g) % 4)
                    if state.get("lastg") is not None:
                        add_dep_helper(i1.ins, state["lastg"], sync=False,
                                       reason="swdge order for queue-sem alignment")
                    i2 = nc.gpsimd.dma_gather(qt[:], dd[t + "_qtab"][:],
                                              dstqw_t[:, g * 32:(g + 1) * 32],
                                              num_idxs=GQ, num_idxs_reg=GQ,
                                              elem_size=640, queue_num=(2 * g + 1) % 4)
                    add_dep_helper(i2.ins, i1.ins, sync=False,
                                   reason="swdge order for queue-sem alignment")
                    state["lastg"] = i2.ins
                    state["q"] = qt
                    et = ep.tile([3, GQ], BF16, tag="eat")
                    nc.sync.dma_start(et[:], dd[t + "_eattrT"][:, g * GQ:(g + 1) * GQ])
                    ps1 = psS.tile([32, GQ], F32, tag="s")
                    nc.tensor.matmul(ps1[:], eew1_t[:], et[:], start=True, stop=True)
                    e1 = ep.tile([32, GQ], BF16, tag="enc1")
                    nc.scalar.activation(e1[:], ps1[:], AF.Relu)
                    ps2 = psS.tile([32, GQ], F32, tag="s")
                    nc.tensor.matmul(ps2[:], eew2_t[:], e1[:], start=True, stop=True)
                    e2 = ep.tile([32, GQ], BF16, tag="enc2")
                    nc.scalar.activation(e2[:], ps2[:], AF.Identity, bias=eeb2_t[:])
                    drt = ep.tile([1, GQ], F32, tag="drt")
                    nc.sync.dma_start(drt[:], dd[t + "_dstlr"][:, g * GQ:(g + 1) * GQ])
                    dB = psS.tile([128, GQ], F32, tag="s")
                    nc.tensor.matmul(dB[:], ones1[:], drt[:], start=True, stop=True)
                    oh4 = ep.tile([128, 4, 128], BF16, tag="oh4", bufs=3)
                    dcs = dstl_t[:, g * 4:(g + 1) * 4]
                    in0 = bass.AP(tensor=dcs.tensor, offset=dcs.offset,
                                  ap=[dcs.ap[0], [dcs.ap[1][0], 4], [0, 128]])
                    in1 = bass.AP(tensor=iota_f.tensor, offset=iota_f.offset,
                                  ap=[iota_f.ap[0], [0, 4], [1, 128]])
                    nc.vector.tensor_tensor(out=oh4[:], in0=in0, in1=in1, op=OP.is_equal)
                    state["kv"], state["e2"] = kvt, e2
                    state["oh4"] = oh4

                for b in range(NB):
                    num_ps = psA.tile([128, D], F32, tag="a")
                    extra_ps = psS.tile([128, 66], F32, tag="s")
                    for cc in range(CB):
                        gidx = b * CB + cc
                        g, sl = gidx // 4, gidx % 4
                        if sl == 0:
                            gather_call(g)
                        kvt, e2 = state["kv"], state["e2"]
                        qt = state["q"]
                        e2c = e2[:, sl * 128:(sl + 1) * 128]
                        tr1 = psS.tile([128, 32], BF16, tag="s")
                        nc.tensor.transpose(tr1[:], e2c[:], ident_bf[:32, :32])
                        e2em = ep.tile([128, 32], BF16, tag="e2em")
                        nc.vector.tensor_copy(e2em[:], tr1[:])
                        oh = state["oh4"][:, sl, :]
                        aqk = ep.tile([128, 2], F32, tag="aqk")
                        aee = ep.tile([128, 2], F32, tag="aee")
                        scr = ep.tile([128, OC], F32, tag="scr")
                        scr2 = ep.tile([128, 32], F32, tag="scr32")
                        for h in range(2):
                            nc.vector.scalar_tensor_tensor(
                                out=scr[:], in0=kvt[:, sl, h * OC:(h + 1) * OC],
                                scalar=0.0625, in1=qt[:, sl, h * OC:(h + 1) * OC],
                                op0=OP.mult, op1=OP.mult,
                                accum_out=aqk[:, h:h + 1])
                            nc.vector.scalar_tensor_tensor(
                                out=scr2[:], in0=e2em[:],
                                scalar=0.0625, in1=qt[:, sl, D + h * 32:D + (h + 1) * 32],
                                op0=OP.mult, op1=OP.mult,
                                accum_out=aee[:, h:h + 1])
                        alpha = ep.tile([128, 2], F32, tag="alpha")
                        nc.vector.scalar_tensor_tensor(
                            out=alpha[:], in0=aqk[:],
                            scalar=lmask_t[:, gidx:gidx + 1], in1=aee[:],
                            op0=OP.add, op1=OP.add)
                        ea = ep.tile([128, 2], F32, tag="ea")
                        nc.scalar.activation(ea[:], alpha[:], AF.Exp)
                        vea = ep.tile([128, D], BF16, tag="vea")
                        combo = ep.tile([128, 66], BF16, tag="combo")
                        for h in range(2):
                            nc.scalar.activation(vea[:, h * OC:(h + 1) * OC],
                                                 kvt[:, sl, D + h * OC:D + (h + 1) * OC],
                                                 AF.Copy, scale=ea[:, h:h + 1])
                            nc.vector.tensor_scalar_mul(
                                combo[:, h * 32:(h + 1) * 32], e2em[:],
                                ea[:, h:h + 1])
                        nc.vector.tensor_copy(combo[:, 64:66], ea[:])
                        nc.tensor.matmul(num_ps[:], oh[:], vea[:],
                                         start=(cc == 0), stop=False)
                        nc.tensor.matmul(extra_ps[:], oh[:], combo[:],
                                         start=(cc == 0), stop=(cc == CB - 1))

                    # ---- block epilogue
                    # fold edge-encoder value term: num += (sum ea*enc2) @ we
                    s_sb = bp.tile([128, 64], BF16, tag="ssb")
                    nc.vector.tensor_copy(s_sb[:], extra_ps[:, 0:64])
                    for h in range(2):
                        tr3 = psS.tile([32, 128], BF16, tag="s")
                        nc.tensor.transpose(tr3[:], s_sb[:, h * 32:(h + 1) * 32],
                                            ident_bf[:])
                        sT = bp.tile([32, 128], BF16, tag="sT")
                        nc.vector.tensor_copy(sT[:], tr3[:])
                        nc.tensor.matmul(num_ps[:, h * OC:(h + 1) * OC],
                                         sT[:],
                                         we_t[:, h * OC:(h + 1) * OC],
                                         start=False, stop=(h == 1))
                    dfix = bp.tile([128, 2], F32, tag="dfix")
                    nc.vector.tensor_scalar_add(dfix[:], extra_ps[:, 64:66], 1e-16)
                    rec = bp.tile([128, 2], F32, tag="rec")
                    nc.vector.reciprocal(rec[:], dfix[:])
                    att = bp.tile([128, OC], F32, tag="att")
                    nc.scalar.activation(att[:], num_ps[:, 0:OC], AF.Copy,
                                         scale=rec[:, 0:1])
                    att1 = bp.tile([128, OC], F32, tag="att1")
                    nc.scalar.activation(att1[:], num_ps[:, OC:D], AF.Copy,
                                         scale=rec[:, 1:2])
                    tsum = bp.tile([128, OC], F32, tag="tsum")
                    nc.vector.tensor_add(tsum[:], att[:], att1[:])
                    u = bp.tile([128, OC], F32, tag="u")
                    nc.vector.scalar_tensor_tensor(
                        out=u[:], in0=tsum[:], scalar=0.5, in1=skips[b][:],
                        op0=OP.mult, op1=OP.add)
                    relu_u = bp.tile([128, OC], F32, tag="relu_u")
                    nc.scalar.activation(relu_u[:], u[:], AF.Relu)
                    negu = bp.tile([128, OC], F32, tag="negu")
                    nc.vector.tensor_scalar_min(negu[:], u[:], 0.0)
                    expn = bp.tile([128, OC], F32, tag="expn")
                    nc.scalar.activation(expn[:], negu[:], AF.Exp)
                    elu = bp.tile([128, OC], F32, tag="elu")
                    nc.vector.scalar_tensor_tensor(
                        out=elu[:], in0=expn[:], scalar=-1.0, in1=relu_u[:],
                        op0=OP.add, op1=OP.add)

                    # ---- proj1
                    h_ps = psS.tile([128, HID], F32, tag="s")
                    for half in range(2):
                        tp = psS.tile([128, 128], F32, tag="s")
                        nc.tensor.transpose(tp[:], elu[:, half * 128:(half + 1) * 128],
                                            ident[:])
                        tps = bp.tile([128, 128], F32, tag="eluT")
                        nc.vector.tensor_copy(tps[:], tp[:])
                        pw = wpool.tile([128, HID], F32, tag="projw")
                        nc.sync.dma_start(pw[:],
                                          dd["proj1w"][half * 128:(half + 1) * 128, :])
                        nc.tensor.matmul(h_ps[:], tps[:], pw[:], start=(half == 0),
                                         stop=False)
                    nc.tensor.matmul(h_ps[:], ones1[:], proj1b_t[:], start=False,
                                     stop=True)
                    h_sb = bp.tile([128, HID], F32, tag="h")
                    nc.vector.tensor_copy(h_sb[:], h_ps[:])

                    # ---- pooling
                    tph = psS.tile([128, 128], F32, tag="s")
                    nc.tensor.transpose(tph[:], h_sb[:], ident[:])
                    hT = bp.tile([128, 128], F32, tag="hT")
                    nc.scalar.copy(hT[:], tph[:])
                    t_ps = psS.tile([128, HID], F32, tag="s")
                    nc.tensor.matmul(t_ps[:], hT[:], poolwq_t[:], start=True, stop=False)
                    nc.tensor.matmul(t_ps[:], ohgT_t[:, b * 128:(b + 1) * 128],
                                     ctxk_sb[:], start=False, stop=True)
                    tanh_sb = bp.tile([128, HID], F32, tag="tanh")
                    nc.scalar.activation(tanh_sb[:], t_ps[:], AF.Tanh)
                    s_t = bp.tile([128, 1], F32, tag="s")
                    scr3 = bp.tile([128, HID], F32, tag="scr3")
                    nc.vector.scalar_tensor_tensor(
                        out=scr3[:], in0=tanh_sb[:], scalar=0.0, in1=poolvB_t[:],
                        op0=OP.bypass, op1=OP.mult, accum_out=s_t[:])
                    es = bp.tile([128, 1], F32, tag="es")
                    nc.scalar.activation(es[:], s_t[:], AF.Exp,
                                         bias=nmask_t[:, b:b + 1])
                    esh = bp.tile([128, HID], F32, tag="esh")
                    nc.scalar.activation(esh[:], h_sb[:], AF.Copy, scale=es[:])
                    pp = psS.tile([128, 4], F32, tag="s")
                    nc.tensor.matmul(pp[:], esh[:], ohg_t[:, b * 4:(b + 1) * 4],
                                     start=True, stop=True)
                    nc.vector.tensor_add(pnum[:], pnum[:], pp[:])
                    ddn = psS.tile([1, 4], F32, tag="s")
                    nc.tensor.matmul(ddn[:], es[:], ohg_t[:, b * 4:(b + 1) * 4],
                                     start=True, stop=True)
                    nc.vector.tensor_add(pden[:], pden[:], ddn[:])

                # ---- pooled = num / den
                dfix2 = fin.tile([1, 4], F32, tag="dfix2" + t)
                nc.vector.tensor_scalar_add(dfix2[:], pden[:], 1e-16)
                rden = fin.tile([1, 4], F32, tag="rden" + t)
                nc.vector.reciprocal(rden[:], dfix2[:])
                bc_ps = psS.tile([128, 4], F32, tag="s")
                nc.tensor.matmul(bc_ps[:], ones1[:], rden[:], start=True, stop=True)
                bc_sb = fin.tile([128, 4], F32, tag="bc" + t)
                nc.vector.tensor_copy(bc_sb[:], bc_ps[:])
                g2 = fin.tile([128, 4], F32, tag="g2" + t)
                nc.vector.tensor_mul(g2[:], pnum[:], bc_sb[:])
                G2[t] = g2

        # ---- final head MLPs
        ones4 = fin.tile([1, 4], F32, tag="ones4")
        nc.vector.memset(ones4[:], 1.0)

        def mlp2(pre, chunks, row_off=0):
            ps = psS.tile([128, 4], F32, tag="s")
            for i, ch in enumerate(chunks):
                wt = wpool.tile([128, HID], F32, tag="mlpw")
                r0 = row_off + i * 128
                nc.sync.dma_start(wt[:], dd[pre + "w1"][r0:r0 + 128, :])
                nc.tensor.matmul(ps[:], wt[:], ch[:], start=(i == 0), stop=False)
            b1 = wtile(pre + "b1", pool=wpool)
            nc.tensor.matmul(ps[:], b1[:], ones4[:], start=False, stop=True)
            rl = fin.tile([128, 4], F32, tag="rl" + pre)
            nc.scalar.activation(rl[:], ps[:], AF.Relu)
            w2 = wtile(pre + "w2", pool=wpool)
            o = psS.tile([1, 4], F32, tag="s")
            nc.tensor.matmul(o[:], w2[:], rl[:], start=True, stop=True)
            ob = fin.tile([1, 4], F32, tag="o" + pre)
            nc.vector.tensor_copy(ob[:], o[:])
            return ob

        gf = [G2["L"], SEQ["L"][:, 0:4], G2["H"], SEQ["H"][:, 0:4],
              G2["A"], SEQ["A"][:, 0:4]]
        dg_o = mlp2("dg", gf)

        ic = []
        for a, bb in [(G2["L"], G2["A"]), (G2["H"], G2["A"])]:
            m = fin.tile([128, 4], F32, tag="im" + str(len(ic)))
            nc.vector.tensor_mul(m[:], a[:], bb[:])
            ic.append(m)
        for a, bb in [(G2["L"], G2["A"]), (G2["H"], G2["A"])]:
            sd = fin.tile([128, 4], F32, tag="isd" + str(len(ic)))
            nc.vector.tensor_sub(sd[:], a[:], bb[:])
            ab = fin.tile([128, 4], F32, tag="iab" + str(len(ic)))
            nc.scalar.activation(ab[:], sd[:], AF.Abs)
            ic.append(ab)
        inter_o = mlp2("inter", ic)

        sdl = []
        for t in types:
            sd = fin.tile([128, 4], F32, tag="sd" + t)
            nc.vector.tensor_sub(sd[:], SEQ[t][:, 0:4], SEQ[t][:, 4:8])
            sdl.append(sd)
        ddg_o = mlp2("ddg", sdl, row_off=384)

        p1 = fin.tile([1, 4], F32, tag="p1")
        nc.vector.tensor_add(p1[:], dg_o[:], inter_o[:])
        p2 = fin.tile([1, 4], F32, tag="p2")
        nc.vector.tensor_add(p2[:], p1[:], ddg_o[:])
        pf = fin.tile([1, 4], F32, tag="pf")
        nc.scalar.activation(pf[:], p2[:], AF.Identity, bias=meta["b2sum"])
        nc.sync.dma_start(pred_d[:], pf[:])

    nc.compile()
    return nc


# ----------------------------------------------------------------------------
# entry point
# ----------------------------------------------------------------------------

def kernel(**inputs):
    meta, in_maps, w = _prep(inputs)
    key = (meta["N"], meta["NB"], meta["CB"], meta["b2sum"])
    if key not in _CACHE:
        _CACHE[key] = _build(meta, w)
    nc = _CACHE[key]
    for m in in_maps:
        for k, v in w.items():
            m["w_" + k] = v
    res = run_bass_kernel_spmd(nc, in_maps, list(range(NCORES)))
    out = np.concatenate([res.results[c]["pred"][0] for c in range(NCORES)])
    return out.astype(np.float32)
